# revision 89
# baseline (speedup 1.0000x reference)
"""Trainium2 Bass kernel for nn_BlockWithCache (Music-Transformer block w/ rel-pos).

Sharding (8 NeuronCores, uniform SPMD program; per-core differences live in the
input data only):
  - core c: batch element b = c//2, tensor-parallel half = c%2.
  - Attention: TP over heads — each core computes its 8 of 16 heads for the
    full 1024-token sequence (weight column slices supplied by the host).
  - Wproj row-slices produce partial attention outputs; a pairwise
    ReduceScatter(add) both completes the sum and splits tokens in half.
  - From the residual on: token-split — each core owns 512 tokens through
    LN2 + FFN (full 4*D hidden) and writes a disjoint output half.

v2 notes:
  - bf16 everywhere on the matmul path; weights host-packed into SBUF tile
    layouts so each matrix loads with 1-3 large contiguous DMAs.
  - Transposed attention: logits are computed as [key, query] (kt stationary,
    qt moving); Srel (read back from the DRAM skew buffer in [q, k] rows) is
    accumulated into the same PSUM block by a matmul with Srel as the
    stationary operand, which transposes it for free.  exp() then writes
    attT directly - no PE transposes and no separate PSUM->SBUF copies.
  - Softmax denominators come from a ones-column appended to V (attV PSUM row
    64); normalization is folded into the yp->ysb copy as a broadcast mult.
  - Skew/negpad DMAs issue from the (otherwise idle) Pool engine, bypassing
    the shared HWDGE descriptor-generation bottleneck.
  - FFN runs in two 256-token halves so FFN2(half A) overlaps FFN1(half B).
"""

import os
import sys

os.environ.setdefault("MYCRO_LOCAL_CACHE", "1")
if "/opt/trn_rl_repo" not in sys.path:
    sys.path.insert(0, "/opt/trn_rl_repo")

import numpy as np

B, L, D, H = 4, 1024, 1024, 16
HS = D // H          # 64
P = 128
TC = L // P          # 8 token chunks
DCH = D // P         # 8 feature chunks
NHC = H // 2         # 8 heads per core
FD = 4 * D           # 4096
FC = FD // P         # 32
TMY = L // 2         # 512 tokens owned after RS
T2 = TMY // P        # 4
EPS = 1e-5
SCALE = 1.0 / 8.0    # 1/sqrt(HS)
NEG = -1.0e9

_PROGRAM_CACHE = {}


def _build_program(flags, no_rs=False):
    import concourse.mybir as mybir
    import concourse.tile as tile
    from concourse import bacc
    from concourse.masks import make_identity

    (aff1, aff2, use_bq, use_bk, use_bv, use_bproj, use_bfc, use_bfc2) = flags

    f32 = mybir.dt.float32
    bf16 = mybir.dt.bfloat16
    fp8 = mybir.dt.float8e4
    AF = mybir.ActivationFunctionType
    ALU = mybir.AluOpType
    AX = mybir.AxisListType

    nc = bacc.Bacc("TRN2", target_bir_lowering=False, debug=False, num_devices=8)

    # Host-packed parameters (already in SBUF tile layout; see kernel()).
    x_in = nc.declare_dram_parameter("x", [P, TC, D], bf16, isOutput=False)
    xmy_in = nc.declare_dram_parameter("x_my", [P, T2, D], bf16, isOutput=False)
    wq_in = nc.declare_dram_parameter("wq", [P, DCH, 512], bf16, isOutput=False)
    wk_in = nc.declare_dram_parameter("wk", [P, DCH, 512], bf16, isOutput=False)
    wv_in = nc.declare_dram_parameter("wv", [P, DCH, 512], bf16, isOutput=False)
    wproj_in = nc.declare_dram_parameter("wproj", [P, 4, D], bf16, isOutput=False)
    ert2_in = nc.declare_dram_parameter("ert2", [P, L], bf16, isOutput=False)
    wfc_in = nc.declare_dram_parameter("wfc", [P, DCH, FD], bf16, isOutput=False)
    wfc2_in = nc.declare_dram_parameter("wfc2", [P, FC, D], bf16, isOutput=False)
    # Always-declared small params (cheap; used only when flags set)
    ln1a_in = nc.declare_dram_parameter("ln1a", [D], f32, isOutput=False)
    ln1b_in = nc.declare_dram_parameter("ln1b", [D], f32, isOutput=False)
    ln2a_in = nc.declare_dram_parameter("ln2a", [D], f32, isOutput=False)
    ln2b_in = nc.declare_dram_parameter("ln2b", [D], f32, isOutput=False)
    bq_in = nc.declare_dram_parameter("bq", [P, 4], f32, isOutput=False)
    bk_in = nc.declare_dram_parameter("bk", [P, 4], f32, isOutput=False)
    bv_in = nc.declare_dram_parameter("bv", [NHC * HS], f32, isOutput=False)
    bproj_in = nc.declare_dram_parameter("bproj", [D], f32, isOutput=False)
    bfc_in = nc.declare_dram_parameter("bfc", [P, FC], f32, isOutput=False)
    bfc2_in = nc.declare_dram_parameter("bfc2", [D], f32, isOutput=False)

    out_dram = nc.declare_dram_parameter("out_my", [TMY, D], f32, isOutput=True)

    def layernorm(tc, nc, pools, xin, hs, nchunks, aff, wbc, bbc, eps_ap):
        """Per-chunk two-pass LN; xin(t) returns the [P, D] f32 input AP,
        hs[t] is the bf16 output tile."""
        small, scratch = pools
        for t in range(nchunks):
            xap = xin(t)
            st = small.tile([P, 8], f32, tag="ln_st")
            # st cols: 0 sum, 1 sumsq, 2 mu, 3 mu^2, 4 var, 5 std, 6 rstd, 7 mur
            nc.vector.reduce_sum(st[:, 0:1], xap, axis=AX.X)
            sq = scratch.tile([P, D], f32, tag="ln_sq")
            nc.scalar.activation(sq[:], xap, AF.Square, accum_out=st[:, 1:2])
            nc.vector.tensor_scalar_mul(st[:, 2:3], st[:, 0:1], 1.0 / D)
            nc.vector.tensor_tensor(st[:, 3:4], st[:, 2:3], st[:, 2:3], op=ALU.mult)
            nc.vector.tensor_scalar(
                st[:, 4:5], st[:, 1:2], 1.0 / D, st[:, 3:4],
                op0=ALU.mult, op1=ALU.subtract,
            )
            nc.scalar.activation(st[:, 5:6], st[:, 4:5], AF.Sqrt, bias=eps_ap)
            nc.vector.reciprocal(st[:, 6:7], st[:, 5:6])
            nc.vector.tensor_tensor(st[:, 7:8], st[:, 2:3], st[:, 6:7], op=ALU.mult)
            nc.vector.tensor_scalar(
                hs[t][:],
                xap,
                st[:, 6:7],
                st[:, 7:8],
                op0=ALU.mult,
                op1=ALU.subtract,
            )
            if aff:
                nc.vector.tensor_tensor(hs[t][:], hs[t][:], wbc[:], op=ALU.mult)
                nc.vector.tensor_tensor(hs[t][:], hs[t][:], bbc[:], op=ALU.add)

    with tile.TileContext(nc) as tc:
        import contextlib

        with contextlib.ExitStack() as es:
            cst = es.enter_context(tc.tile_pool(name="cst", bufs=1))
            small = es.enter_context(tc.tile_pool(name="small", bufs=2))
            dram = es.enter_context(tc.tile_pool(name="dram", bufs=1, space="DRAM"))

            eps_t = cst.tile([P, 1], f32)
            nc.vector.memset(eps_t[:], EPS)
            warm = cst.tile([P, 2], f32)
            nc.vector.memset(warm[:], 1.0)
            for fn in (AF.Square, AF.Sqrt, AF.Exp, AF.Gelu, AF.Copy):
                nc.scalar.activation(warm[:, 1:2], warm[:, 0:1], fn)
            id16 = cst.tile([P, P], bf16)
            make_identity(nc, id16)
            ert2 = cst.tile([P, L], bf16)
            nc.sync.dma_start(ert2[:], ert2_in[:])

            ln1w_bc = ln1b_bc = ln2w_bc = ln2b_bc = None
            if aff1:
                row = cst.tile([1, D], f32, tag="lnrow1a")
                nc.sync.dma_start(row[:], ln1a_in[None, :])
                ln1w_bc = cst.tile([P, D], f32)
                nc.gpsimd.partition_broadcast(ln1w_bc[:], row[:])
                row2 = cst.tile([1, D], f32, tag="lnrow1b")
                nc.sync.dma_start(row2[:], ln1b_in[None, :])
                ln1b_bc = cst.tile([P, D], f32)
                nc.gpsimd.partition_broadcast(ln1b_bc[:], row2[:])
            if aff2:
                row = cst.tile([1, D], f32, tag="lnrow2a")
                nc.sync.dma_start(row[:], ln2a_in[None, :])
                ln2w_bc = cst.tile([P, D], f32)
                nc.gpsimd.partition_broadcast(ln2w_bc[:], row[:])
                row2 = cst.tile([1, D], f32, tag="lnrow2b")
                nc.sync.dma_start(row2[:], ln2b_in[None, :])
                ln2b_bc = cst.tile([P, D], f32)
                nc.gpsimd.partition_broadcast(ln2b_bc[:], row2[:])
            bq_sb = bk_sb = None
            if use_bq:
                bq_sb = cst.tile([P, 4], f32)
                nc.sync.dma_start(bq_sb[:], bq_in[:])
            if use_bk:
                bk_sb = cst.tile([P, 4], f32)
                nc.sync.dma_start(bk_sb[:], bk_in[:])
            bv_bc = None
            if use_bv:
                row = cst.tile([1, NHC * HS], f32, tag="bvrow")
                nc.sync.dma_start(row[:], bv_in[None, :])
                bv_bc = cst.tile([P, NHC * HS], f32)
                nc.gpsimd.partition_broadcast(bv_bc[:], row[:])
            bproj_bc = None
            if use_bproj:
                row = cst.tile([1, D], f32, tag="bprow")
                nc.sync.dma_start(row[:], bproj_in[None, :])
                bproj_bc = cst.tile([P, D], f32)
                nc.gpsimd.partition_broadcast(bproj_bc[:], row[:])
            bfc_sb = None
            if use_bfc:
                bfc_sb = cst.tile([P, FC], f32)
                nc.sync.dma_start(bfc_sb[:], bfc_in[:])
            bfc2_bc = None
            if use_bfc2:
                row = cst.tile([1, D], f32, tag="b2row")
                nc.sync.dma_start(row[:], bfc2_in[None, :])
                bfc2_bc = cst.tile([P, D], f32)
                nc.gpsimd.partition_broadcast(bfc2_bc[:], row[:])

            # Skew DRAM buffers: per (qc, pr-parity): [2 slots][128 rows][srow]
            # bf16.  Write rows at stride srow, read back at stride srow-1 =>
            # row q is realigned by (127 - q); pad cols [wp, wp+128) hold NEG
            # so the causal mask comes back for free.
            negpad = cst.tile([P, 2, P], bf16)
            nc.vector.memset(negpad[:], NEG)
            skewbufs = []
            for qc in range(TC):
                srow = P * (qc + 2)
                slots = []
                for par in range(2):
                    d1 = dram.tile([2 * P * srow], bf16, name=f"skew_{qc}_{par}")
                    slots.append(d1)
                skewbufs.append(slots)

            def emit_negpads():
                for qc in range(TC):
                    srow = P * (qc + 2)
                    wp = P * (qc + 1)
                    for par in range(2):
                        d1 = skewbufs[qc][par]
                        wv_full = d1[:].rearrange(
                            "(s q c) -> q s c", s=2, q=P, c=srow
                        )
                        nc.sync.dma_start(wv_full[:, :, wp : wp + P], negpad[:])

            # ---------------- persistent activation tiles ----------------
            xmyp = es.enter_context(tc.tile_pool(name="xmyp", bufs=1))
            xmy = xmyp.tile([P, T2, D], bf16)
            wfcq_pool = es.enter_context(tc.tile_pool(name="wfcq", bufs=1))
            wqts = [
                wfcq_pool.tile([P, DCH, 1024], bf16, tag=f"wfcq{q % 2}", name=f"wqt{q}")
                for q in range(4)
            ]
            ysb_pool = tc.alloc_tile_pool(name="ysb", bufs=1)
            ysb = ysb_pool.tile([P, 4, L], bf16)

            qkv_pool = tc.alloc_tile_pool(name="qkv", bufs=1)
            qt_sb = [qkv_pool.tile([P, L], bf16, name=f"qt{p}") for p in range(4)]
            kt_sb = [qkv_pool.tile([P, L], bf16, name=f"kt{p}") for p in range(4)]
            # V with a ones column per head: [:, h, 0:64] = V, [:, h, 64] = 1
            v_sb = [qkv_pool.tile([P, NHC, HS + 1], bf16, name=f"v{t}") for t in range(TC)]

            # ---------------- LN1 + transpose + QKV ----------------
            with tc.tile_pool(name="xp", bufs=1) as xph, tc.tile_pool(
                name="hTp", bufs=1
            ) as hTp:
                xs = xph.tile([P, TC, D], bf16)
                for lo, hi in ((0, 1), (1, 2), (2, 4), (4, 6), (6, 8)):
                    nc.sync.dma_start(xs[:, lo:hi, :], x_in[:, lo:hi, :])
                hTT = hTp.tile([P, DCH, L], bf16)
                with tc.tile_pool(name="xh", bufs=1) as xh, tc.tile_pool(
                    name="lnscr", bufs=3
                ) as lnscr:
                    hs = [xh.tile([P, D], bf16, name=f"h{t}") for t in range(TC)]
                    layernorm(
                        tc, nc, (small, lnscr), lambda t: xs[:, t, :], hs, TC,
                        aff1, ln1w_bc, ln1b_bc, eps_t[:],
                    )
                    with tc.tile_pool(name="htps", bufs=3, space="PSUM") as htps:
                        for t in range(TC):
                            tp = htps.tile([P, DCH, P], bf16, tag="htp")
                            for d in range(DCH):
                                nc.tensor.transpose(
                                    tp[:, d, :], hs[t][:, d * P : (d + 1) * P], id16[:]
                                )
                            nc.any.tensor_copy(hTT[:, :, t * P : (t + 1) * P], tp[:])

                # QKV projections (h freed; hTT alive)
                with tc.tile_pool(name="wqkv", bufs=1) as wp_pool, tc.tile_pool(
                    name="qkvps", bufs=5, space="PSUM"
                ) as qps:
                    wq_sb = wp_pool.tile([P, DCH, 512], bf16)
                    wk_sb = wp_pool.tile([P, DCH, 512], bf16)
                    wv_sb = wp_pool.tile([P, DCH, 512], bf16)
                    nc.sync.dma_start(wq_sb[:], wq_in[:])
                    nc.sync.dma_start(wk_sb[:], wk_in[:])
                    nc.sync.dma_start(wv_sb[:], wv_in[:])
                    emit_negpads()
                    # Q^T and K^T: out [128(2 heads), tokens]
                    for p in range(4):
                        for n in range(2):
                            ps = qps.tile([P, 512], f32, tag="qkvp")
                            for d in range(DCH):
                                nc.tensor.matmul(
                                    ps[:],
                                    wq_sb[:, d, p * P : (p + 1) * P],
                                    hTT[:, d, n * 512 : (n + 1) * 512],
                                    start=(d == 0),
                                    stop=(d == DCH - 1),
                                )
                            nc.any.tensor_copy(
                                qt_sb[p][:, n * 512 : (n + 1) * 512], ps[:]
                            )
                            if use_bq:
                                nc.vector.tensor_scalar_add(
                                    qt_sb[p][:, n * 512 : (n + 1) * 512],
                                    qt_sb[p][:, n * 512 : (n + 1) * 512],
                                    bq_sb[:, p : p + 1],
                                )
                        for n in range(2):
                            ps = qps.tile([P, 512], f32, tag="qkvp")
                            for d in range(DCH):
                                nc.tensor.matmul(
                                    ps[:],
                                    wk_sb[:, d, p * P : (p + 1) * P],
                                    hTT[:, d, n * 512 : (n + 1) * 512],
                                    start=(d == 0),
                                    stop=(d == DCH - 1),
                                )
                            nc.any.tensor_copy(
                                kt_sb[p][:, n * 512 : (n + 1) * 512], ps[:]
                            )
                            if use_bk:
                                nc.vector.tensor_scalar_add(
                                    kt_sb[p][:, n * 512 : (n + 1) * 512],
                                    kt_sb[p][:, n * 512 : (n + 1) * 512],
                                    bk_sb[:, p : p + 1],
                                )
                    # V: out [tokens, 512 hs-cols] -> strided into v_sb + ones
                    for t in range(TC):
                        ps = qps.tile([P, 512], f32, tag="qkvp")
                        for d in range(DCH):
                            nc.tensor.matmul(
                                ps[:],
                                hTT[:, d, t * P : (t + 1) * P],
                                wv_sb[:, d, :],
                                start=(d == 0),
                                stop=(d == DCH - 1),
                            )
                        if use_bv:
                            nc.vector.tensor_tensor(
                                ps[:], ps[:], bv_bc[:], op=ALU.add
                            )
                        nc.any.tensor_copy(v_sb[t][:, :, 0:HS], ps[:])
                        nc.vector.memset(v_sb[t][:, :, HS : HS + 1], 1.0)

            # ---------------- attention (transposed logits) ----------------
            nc.scalar.dma_start(xmy[:], xmy_in[:])
            for q in range(2):
                nc.scalar.dma_start(
                    wqts[q][:], wfc_in[:, :, q * 1024 : (q + 1) * 1024]
                )
            with contextlib.ExitStack() as att_es:
                srelp = att_es.enter_context(tc.tile_pool(name="srelp", bufs=2))
                rsbp = att_es.enter_context(tc.tile_pool(name="rsbp", bufs=4))
                attTp = att_es.enter_context(tc.tile_pool(name="attTp", bufs=3))
                y1p = att_es.enter_context(tc.tile_pool(name="y1p", bufs=3))
                nrmp = att_es.enter_context(tc.tile_pool(name="nrmp", bufs=4))
                lps = att_es.enter_context(tc.tile_pool(name="lps", bufs=3, space="PSUM"))
                rps = att_es.enter_context(tc.tile_pool(name="rps", bufs=3, space="PSUM"))
                yps = att_es.enter_context(tc.tile_pool(name="yps", bufs=2, space="PSUM"))

                def emit_rphase(pr):
                    """R = Q Er^T -> DRAM skew write -> skewed read (Srel).
                    Both heads (slots) of the pair in one pass."""
                    srels = []
                    for qc in range(TC):
                        wp = P * (qc + 1)
                        m0 = 896 - P * qc
                        srow = P * (qc + 2)
                        nsub = (wp + 511) // 512
                        d1 = skewbufs[qc][pr % 2]
                        wview = d1[:].rearrange("(s q c) -> q s c", s=2, q=P, c=srow)
                        rsb = rsbp.tile([P, 2, wp], bf16, tag="rsb")
                        for i in range(2):
                            off = i * HS
                            lhsq = qt_sb[pr][off : off + HS, qc * P : (qc + 1) * P]
                            for s in range(nsub):
                                w = min(512, wp - s * 512)
                                rp = rps.tile([P, 512], f32, tag="rp")
                                nc.tensor.matmul(
                                    rp[:, :w],
                                    lhsq,
                                    ert2[off : off + HS, m0 + s * 512 : m0 + s * 512 + w],
                                    start=True,
                                    stop=True,
                                )
                                nc.vector.tensor_copy(
                                    rsb[:, i, s * 512 : s * 512 + w], rp[:, :w]
                                )
                        nc.gpsimd.dma_start(wview[:, :, :wp], rsb[:])
                        srel = srelp.tile([P, 2, wp], bf16, tag=f"srel{qc}")
                        for i in range(2):
                            rv = d1[i * P * srow + 127 : i * P * srow + 127 + P * (srow - 1)]
                            rview = rv.rearrange("(q c) -> q c", c=srow - 1)
                            nc.sync.dma_start(srel[:, i, :], rview[:, :wp])
                        srels.append(srel)
                    return srels

                srel_pending = {0: emit_rphase(0)}
                for pr in range(4):
                    if pr + 1 < 4:
                        srel_pending[pr + 1] = emit_rphase(pr + 1)
                    srels2 = srel_pending.pop(pr)
                    attT2 = [
                        attTp.tile([P, TC, L], bf16, tag="attT", name=f"attT_{pr}_{i}")
                        for i in range(2)
                    ]
                    # logits^T blocks + exp
                    for qc in range(TC):
                        for i in range(2):
                            off = i * HS
                            qmov = qt_sb[pr][off : off + HS, qc * P : (qc + 1) * P]
                            for cg in range(0, qc + 1, 4):
                                ncc = min(4, qc + 1 - cg)
                                lt = lps.tile([P, 4, P], f32, tag="lt")
                                for j in range(ncc):
                                    cc = cg + j
                                    nc.tensor.matmul(
                                        lt[:, j, :],
                                        kt_sb[pr][off : off + HS, cc * P : (cc + 1) * P],
                                        qmov,
                                        start=True,
                                        stop=False,
                                    )
                                    nc.tensor.matmul(
                                        lt[:, j, :],
                                        srels2[qc][:, i, cc * P : (cc + 1) * P],
                                        id16[:],
                                        start=False,
                                        stop=True,
                                    )
                                nc.scalar.activation(
                                    attT2[i][:, cg : cg + ncc, qc * P : (qc + 1) * P],
                                    lt[:, 0:ncc, :],
                                    AF.Exp,
                                )
                    # att @ V with ones-column -> y rows 0..63, denom row 64
                    h0 = 2 * pr
                    for i in range(2):
                        h = h0 + i
                        for n in range(2):
                            yp = yps.tile([P, 512], f32, tag="yp")
                            ccmax = min(TC, 4 * (n + 1))
                            for cc in range(ccmax):
                                lo = max(n * 512, cc * P)
                                w = (n + 1) * 512 - lo
                                nc.tensor.matmul(
                                    yp[0 : HS + 1, lo - n * 512 : lo - n * 512 + w],
                                    v_sb[cc][:, h, :],
                                    attT2[i][:, cc, lo : lo + w],
                                    start=(cc == 0),
                                    stop=(cc == ccmax - 1),
                                )
                            # normalize: rows 0..63 * (1 / row 64)
                            rcp = nrmp.tile([P, 512], f32, tag="rcp")
                            nrm = nrmp.tile([P, 512], f32, tag="nrm")
                            nc.vector.reciprocal(rcp[0:1, :], yp[HS : HS + 1, :])
                            nc.gpsimd.partition_broadcast(nrm[0:HS, :], rcp[0:1, :])
                            if i == 1:
                                nc.vector.tensor_tensor(
                                    ysb[0:HS, pr, n * 512 : (n + 1) * 512],
                                    yp[0:HS, :],
                                    nrm[0:HS, :],
                                    op=ALU.mult,
                                )
                            else:
                                y1 = y1p.tile([P, 512], bf16, tag="y1")
                                nc.vector.tensor_tensor(
                                    y1[0:HS, :],
                                    yp[0:HS, :],
                                    nrm[0:HS, :],
                                    op=ALU.mult,
                                )
                                nc.gpsimd.dma_start(
                                    ysb[HS:P, pr, n * 512 : (n + 1) * 512],
                                    y1[0:HS, :],
                                )

            qkv_pool.release()

            # Internal DRAM for the pairwise ReduceScatter
            cc_in = [dram.tile([L, 512], bf16, name=f"cc_in{n}") for n in range(2)]
            cc_out = [dram.tile([TMY, 512], bf16, name=f"cc_out{n}") for n in range(2)]

            # ---------------- proj (partial) + ReduceScatter ----------------
            with tc.tile_pool(name="wproj", bufs=1) as wpp, tc.tile_pool(
                name="asb", bufs=1
            ) as asbp, tc.tile_pool(name="aps", bufs=4, space="PSUM") as apsp:
                wproj_sb = wpp.tile([P, 4, D], bf16)
                nc.scalar.dma_start(wproj_sb[:], wproj_in[:])
                for n in range(2):
                    asb = asbp.tile([P, TC, 512], bf16, tag=f"asb{n}")
                    for t in range(TC):
                        ap_ = apsp.tile([P, 512], f32, tag="ap")
                        for p in range(4):
                            nc.tensor.matmul(
                                ap_[:],
                                ysb[:, p, t * P : (t + 1) * P],
                                wproj_sb[:, p, n * 512 : (n + 1) * 512],
                                start=(p == 0),
                                stop=(p == 3),
                            )
                        nc.any.tensor_copy(asb[:, t, :], ap_[:])
                        if t % 2 == 1:
                            nc.sync.dma_start(
                                cc_in[n][:].rearrange("(t q) c -> q t c", q=P)[
                                    :, t - 1 : t + 1, :
                                ],
                                asb[:, t - 1 : t + 1, :],
                            )
                    # fire the column-half collective as soon as its inputs
                    # are written; the other half's matmuls overlap it
                    if no_rs:
                        nc.sync.dma_start(cc_out[n][:], cc_in[n][:TMY, :])
                    else:
                        nc.gpsimd.collective_compute(
                            "ReduceScatter",
                            mybir.AluOpType.add,
                            replica_groups=[[0, 1], [2, 3], [4, 5], [6, 7]],
                            ins=[cc_in[n][:]],
                            outs=[cc_out[n][:]],
                        )
            ysb_pool.release()

            # ---------------- residual + LN2 + h2T ----------------
            x2p = es.enter_context(tc.tile_pool(name="x2p", bufs=1))
            x2 = [x2p.tile([P, D], f32, name=f"x2_{t}") for t in range(T2)]
            h2Tp = es.enter_context(tc.tile_pool(name="h2Tp", bufs=1))
            h2TT = h2Tp.tile([P, DCH, TMY], bf16)
            with tc.tile_pool(name="res", bufs=1) as resp, tc.tile_pool(
                name="lnscr2", bufs=2
            ) as lnscr2:
                arb = resp.tile([P, T2, 2, 512], bf16, tag="arb")
                for n in range(2):
                    for g in range(2):
                        nc.sync.dma_start(
                            arb[:, 2 * g : 2 * g + 2, n, :],
                            cc_out[n][:].rearrange("(t q) c -> q t c", q=P)[
                                :, 2 * g : 2 * g + 2, :
                            ],
                        )
                h2 = [resp.tile([P, D], bf16, name=f"h2_{t}") for t in range(T2)]
                for t in range(T2):
                    nc.vector.tensor_tensor(
                        x2[t][:], xmy[:, t, :], arb[:, t, :, :], op=ALU.add
                    )
                    if use_bproj:
                        nc.vector.tensor_tensor(
                            x2[t][:], x2[t][:], bproj_bc[:], op=ALU.add
                        )
                layernorm(
                    tc, nc, (small, lnscr2), lambda t: x2[t][:], h2, T2,
                    aff2, ln2w_bc, ln2b_bc, eps_t[:],
                )
                with tc.tile_pool(name="h2ps", bufs=2, space="PSUM") as h2ps:
                    for t in range(T2):
                        tp = h2ps.tile([P, DCH, P], bf16, tag="h2p")
                        for d in range(DCH):
                            nc.tensor.transpose(
                                tp[:, d, :], h2[t][:, d * P : (d + 1) * P], id16[:]
                            )
                        nc.any.tensor_copy(h2TT[:, :, t * P : (t + 1) * P], tp[:])

            # ---------------- FFN (f-streamed; FFN2 n=0 rides FFN1) ----------
            # FFN1 produces m1T[:, f, :] per f-chunk; FFN2's n=0 column half
            # accumulates in PSUM as each chunk lands, so PE stays dense.  The
            # n=1 half runs as a second f-pass from the kept m1T.
            m1p = es.enter_context(tc.tile_pool(name="m1p", bufs=1))
            m1T = m1p.tile([P, FC, TMY], bf16)
            outp = es.enter_context(tc.tile_pool(name="outp", bufs=1))
            out_sb = outp.tile([P, T2, D], f32)
            with tc.tile_pool(name="wfc2p", bufs=1) as wfc2_pool, tc.tile_pool(
                name="fc1ps", bufs=2, space="PSUM"
            ) as fc1ps, tc.tile_pool(name="fc2ps", bufs=4, space="PSUM") as fc2ps:
                w2n = [
                    wfc2_pool.tile([P, FC, 512], bf16, tag=f"w2n{n}", name=f"w2n{n}")
                    for n in range(2)
                ]
                nc.scalar.dma_start(w2n[0][:], wfc2_in[:, :, 0:512])
                nc.sync.dma_start(w2n[1][:], wfc2_in[:, :, 512:1024])
                pss = [fc2ps.tile([P, 512], f32, tag="fc2", name=f"fc2a_{t}") for t in range(T2)]
                for q in range(4):
                    wq_t = wqts[q]
                    for fl in range(DCH):
                        f = q * DCH + fl
                        mp = fc1ps.tile([P, TMY], f32, tag="m1ps")
                        for d in range(DCH):
                            nc.tensor.matmul(
                                mp[:],
                                wq_t[:, d, fl * P : (fl + 1) * P],
                                h2TT[:, d, :],
                                start=(d == 0),
                                stop=(d == DCH - 1),
                            )
                        if use_bfc:
                            nc.scalar.activation(
                                m1T[:, f, :], mp[:], AF.Gelu,
                                bias=bfc_sb[:, f : f + 1],
                            )
                        else:
                            nc.scalar.activation(m1T[:, f, :], mp[:], AF.Gelu)
                        for t in range(T2):
                            nc.tensor.matmul(
                                pss[t][:],
                                m1T[:, f, t * P : (t + 1) * P],
                                w2n[0][:, f, :],
                                start=(f == 0),
                                stop=(f == FC - 1),
                            )
                    if q + 2 < 4:
                        nc.scalar.dma_start(
                            wqts[q + 2][:],
                            wfc_in[:, :, (q + 2) * 1024 : (q + 3) * 1024],
                        )
                for t in range(T2):
                    nc.vector.tensor_tensor(
                        out_sb[:, t, 0:512],
                        pss[t][:],
                        x2[t][:, 0:512],
                        op=ALU.add,
                    )
                # second pass: n=1 column half, t-major so the tail pipelines
                outv = out_dram[:].rearrange("(t q) c -> q t c", q=P)
                for t in range(T2):
                    ps2 = fc2ps.tile([P, 512], f32, tag="fc2", name=f"fc2b_{t}")
                    for f in range(FC):
                        nc.tensor.matmul(
                            ps2[:],
                            m1T[:, f, t * P : (t + 1) * P],
                            w2n[1][:, f, :],
                            start=(f == 0),
                            stop=(f == FC - 1),
                        )
                    nc.vector.tensor_tensor(
                        out_sb[:, t, 512:1024],
                        ps2[:],
                        x2[t][:, 512:1024],
                        op=ALU.add,
                    )
                    if use_bfc2:
                        nc.vector.tensor_tensor(
                            out_sb[:, t, :], out_sb[:, t, :], bfc2_bc[:],
                            op=ALU.add,
                        )
                    nc.sync.dma_start(outv[:, t, :], out_sb[:, t, :])

    nc.compile()
    return nc


def _get_program(flags):
    if flags not in _PROGRAM_CACHE:
        _PROGRAM_CACHE[flags] = _build_program(flags)
    return _PROGRAM_CACHE[flags]


def kernel(
    x,
    ln1_w,
    ln1_b,
    Wqkv,
    bqkv,
    Wproj,
    bproj,
    Er,
    ln2_w,
    ln2_b,
    Wfc,
    bfc,
    Wfc2,
    bfc2,
):
    import ml_dtypes
    from concourse.bass_utils import run_bass_kernel_spmd

    bf = ml_dtypes.bfloat16
    x = np.asarray(x, np.float32)
    f = np.float32
    ntriv = lambda a, v: not np.all(np.asarray(a) == v)
    flags = (
        ntriv(ln1_w, 1) or ntriv(ln1_b, 0),
        ntriv(ln2_w, 1) or ntriv(ln2_b, 0),
        ntriv(bqkv[:D], 0),
        ntriv(bqkv[D : 2 * D], 0),
        ntriv(bqkv[2 * D :], 0),
        ntriv(bproj, 0),
        ntriv(bfc, 0),
        ntriv(bfc2, 0),
    )
    nc = _get_program(flags)

    c = np.ascontiguousarray

    def pack_w(m, nch):
        # [rows, cols] -> [128, nch, cols] where rows = nch*128 chunk-major
        m = np.asarray(m, f)
        rows, cols = m.shape
        return c(m.reshape(nch, P, cols).transpose(1, 0, 2).astype(bf))

    ert2_f = np.concatenate([np.asarray(Er, f).T, np.asarray(Er, f).T], axis=0)
    ert2_pk = c(ert2_f.astype(bf))
    wfc_pk = pack_w(np.asarray(Wfc), DCH)
    wfc2_pk = pack_w(np.asarray(Wfc2), FC)

    in_maps = []
    for core in range(8):
        b, half = divmod(core, 2)
        hs0, hs1 = half * 512, (half + 1) * 512
        bq = np.asarray(bqkv[:D][hs0:hs1], f) * SCALE
        bk = np.asarray(bqkv[D : 2 * D][hs0:hs1], f)
        wq = np.asarray(Wqkv)[:, 0:D][:, hs0:hs1] * SCALE
        x_r = x[b].reshape(TC, P, D)
        x_pk = c(x_r.transpose(1, 0, 2).astype(bf))
        xmy_pk = c(x_r[half * T2 : (half + 1) * T2].transpose(1, 0, 2).astype(bf))
        in_maps.append(
            {
                "x": x_pk,
                "x_my": xmy_pk,
                "wq": pack_w(wq, DCH),
                "wk": pack_w(np.asarray(Wqkv)[:, D : 2 * D][:, hs0:hs1], DCH),
                "wv": pack_w(np.asarray(Wqkv)[:, 2 * D :][:, hs0:hs1], DCH),
                "wproj": pack_w(np.asarray(Wproj)[hs0:hs1, :].reshape(4, 2, HS, D)[:, ::-1].reshape(512, D), 4),
                "ert2": ert2_pk,
                "wfc": wfc_pk,
                "wfc2": wfc2_pk,
                "ln1a": c(np.asarray(ln1_w), f),
                "ln1b": c(np.asarray(ln1_b), f),
                "ln2a": c(np.asarray(ln2_w), f),
                "ln2b": c(np.asarray(ln2_b), f),
                "bq": c(bq.reshape(4, P).T, f),
                "bk": c(bk.reshape(4, P).T, f),
                "bv": c(np.asarray(bqkv[2 * D :][hs0:hs1]), f),
                "bproj": c(np.asarray(bproj), f),
                "bfc": c(np.asarray(bfc).reshape(FC, P).T, f),
                "bfc2": c(np.asarray(bfc2), f),
            }
        )

    trace = bool(int(os.environ.get("KERNEL_TRACE", "0")))
    res = run_bass_kernel_spmd(nc, in_maps, list(range(8)), trace=trace)
    global LAST_EXEC_NS, LAST_RESULT
    LAST_EXEC_NS = res.exec_time_ns
    LAST_RESULT = res
    out = np.empty((B, L, D), np.float32)
    for core in range(8):
        b, half = divmod(core, 2)
        out[b, half * 512 : (half + 1) * 512] = res.results[core]["out_my"]
    return out


LAST_EXEC_NS = None
LAST_RESULT = None


# revision 90
# speedup vs baseline: 1.0083x; 1.0083x over previous
"""Trainium2 Bass kernel for nn_BlockWithCache (Music-Transformer block w/ rel-pos).

Sharding (8 NeuronCores, uniform SPMD program; per-core differences live in the
input data only):
  - core c: batch element b = c//2, tensor-parallel half = c%2.
  - Attention: TP over heads — each core computes its 8 of 16 heads for the
    full 1024-token sequence (weight column slices supplied by the host).
  - Wproj row-slices produce partial attention outputs; a pairwise
    ReduceScatter(add) both completes the sum and splits tokens in half.
  - From the residual on: token-split — each core owns 512 tokens through
    LN2 + FFN (full 4*D hidden) and writes a disjoint output half.

v2 notes:
  - bf16 everywhere on the matmul path; weights host-packed into SBUF tile
    layouts so each matrix loads with 1-3 large contiguous DMAs.
  - Transposed attention: logits are computed as [key, query] (kt stationary,
    qt moving); Srel (read back from the DRAM skew buffer in [q, k] rows) is
    accumulated into the same PSUM block by a matmul with Srel as the
    stationary operand, which transposes it for free.  exp() then writes
    attT directly - no PE transposes and no separate PSUM->SBUF copies.
  - Softmax denominators come from a ones-column appended to V (attV PSUM row
    64); normalization is folded into the yp->ysb copy as a broadcast mult.
  - Skew/negpad DMAs issue from the (otherwise idle) Pool engine, bypassing
    the shared HWDGE descriptor-generation bottleneck.
  - FFN runs in two 256-token halves so FFN2(half A) overlaps FFN1(half B).
"""

import os
import sys

os.environ.setdefault("MYCRO_LOCAL_CACHE", "1")
if "/opt/trn_rl_repo" not in sys.path:
    sys.path.insert(0, "/opt/trn_rl_repo")

import numpy as np

B, L, D, H = 4, 1024, 1024, 16
HS = D // H          # 64
P = 128
TC = L // P          # 8 token chunks
DCH = D // P         # 8 feature chunks
NHC = H // 2         # 8 heads per core
FD = 4 * D           # 4096
FC = FD // P         # 32
TMY = L // 2         # 512 tokens owned after RS
T2 = TMY // P        # 4
EPS = 1e-5
SCALE = 1.0 / 8.0    # 1/sqrt(HS)
NEG = -1.0e9

_PROGRAM_CACHE = {}


def _build_program(flags, no_rs=False):
    import concourse.mybir as mybir
    import concourse.tile as tile
    from concourse import bacc
    from concourse.masks import make_identity

    (aff1, aff2, use_bq, use_bk, use_bv, use_bproj, use_bfc, use_bfc2) = flags

    f32 = mybir.dt.float32
    bf16 = mybir.dt.bfloat16
    fp8 = mybir.dt.float8e4
    AF = mybir.ActivationFunctionType
    ALU = mybir.AluOpType
    AX = mybir.AxisListType

    nc = bacc.Bacc("TRN2", target_bir_lowering=False, debug=False, num_devices=8)

    # Host-packed parameters (already in SBUF tile layout; see kernel()).
    x_in = nc.declare_dram_parameter("x", [P, TC, D], bf16, isOutput=False)
    xmy_in = nc.declare_dram_parameter("x_my", [P, T2, D], bf16, isOutput=False)
    wq_in = nc.declare_dram_parameter("wq", [P, DCH, 512], bf16, isOutput=False)
    wk_in = nc.declare_dram_parameter("wk", [P, DCH, 512], bf16, isOutput=False)
    wv_in = nc.declare_dram_parameter("wv", [P, DCH, 512], bf16, isOutput=False)
    wproj_in = nc.declare_dram_parameter("wproj", [P, 4, D], bf16, isOutput=False)
    ert2_in = nc.declare_dram_parameter("ert2", [P, L], bf16, isOutput=False)
    wfc_in = nc.declare_dram_parameter("wfc", [P, DCH, FD], bf16, isOutput=False)
    wfc2_in = nc.declare_dram_parameter("wfc2", [P, FC, D], bf16, isOutput=False)
    # Always-declared small params (cheap; used only when flags set)
    ln1a_in = nc.declare_dram_parameter("ln1a", [D], f32, isOutput=False)
    ln1b_in = nc.declare_dram_parameter("ln1b", [D], f32, isOutput=False)
    ln2a_in = nc.declare_dram_parameter("ln2a", [D], f32, isOutput=False)
    ln2b_in = nc.declare_dram_parameter("ln2b", [D], f32, isOutput=False)
    bq_in = nc.declare_dram_parameter("bq", [P, 4], f32, isOutput=False)
    bk_in = nc.declare_dram_parameter("bk", [P, 4], f32, isOutput=False)
    bv_in = nc.declare_dram_parameter("bv", [NHC * HS], f32, isOutput=False)
    bproj_in = nc.declare_dram_parameter("bproj", [D], f32, isOutput=False)
    bfc_in = nc.declare_dram_parameter("bfc", [P, FC], f32, isOutput=False)
    bfc2_in = nc.declare_dram_parameter("bfc2", [D], f32, isOutput=False)

    out_dram = nc.declare_dram_parameter("out_my", [TMY, D], f32, isOutput=True)

    def layernorm(tc, nc, pools, xin, hs, nchunks, aff, wbc, bbc, eps_ap):
        """Per-chunk two-pass LN; xin(t) returns the [P, D] f32 input AP,
        hs[t] is the bf16 output tile."""
        small, scratch = pools
        for t in range(nchunks):
            xap = xin(t)
            st = small.tile([P, 8], f32, tag="ln_st")
            # st cols: 0 sum, 1 sumsq, 2 mu, 3 mu^2, 4 var, 5 std, 6 rstd, 7 mur
            nc.vector.reduce_sum(st[:, 0:1], xap, axis=AX.X)
            sq = scratch.tile([P, D], f32, tag="ln_sq")
            nc.scalar.activation(sq[:], xap, AF.Square, accum_out=st[:, 1:2])
            nc.vector.tensor_scalar_mul(st[:, 2:3], st[:, 0:1], 1.0 / D)
            nc.vector.tensor_tensor(st[:, 3:4], st[:, 2:3], st[:, 2:3], op=ALU.mult)
            nc.vector.tensor_scalar(
                st[:, 4:5], st[:, 1:2], 1.0 / D, st[:, 3:4],
                op0=ALU.mult, op1=ALU.subtract,
            )
            nc.scalar.activation(st[:, 5:6], st[:, 4:5], AF.Sqrt, bias=eps_ap)
            nc.vector.reciprocal(st[:, 6:7], st[:, 5:6])
            nc.vector.tensor_tensor(st[:, 7:8], st[:, 2:3], st[:, 6:7], op=ALU.mult)
            nc.vector.tensor_scalar(
                hs[t][:],
                xap,
                st[:, 6:7],
                st[:, 7:8],
                op0=ALU.mult,
                op1=ALU.subtract,
            )
            if aff:
                nc.vector.tensor_tensor(hs[t][:], hs[t][:], wbc[:], op=ALU.mult)
                nc.vector.tensor_tensor(hs[t][:], hs[t][:], bbc[:], op=ALU.add)

    with tile.TileContext(nc) as tc:
        import contextlib

        with contextlib.ExitStack() as es:
            cst = es.enter_context(tc.tile_pool(name="cst", bufs=1))
            small = es.enter_context(tc.tile_pool(name="small", bufs=2))
            dram = es.enter_context(tc.tile_pool(name="dram", bufs=1, space="DRAM"))

            eps_t = cst.tile([P, 1], f32)
            nc.vector.memset(eps_t[:], EPS)
            warm = cst.tile([P, 2], f32)
            nc.vector.memset(warm[:], 1.0)
            for fn in (AF.Square, AF.Sqrt, AF.Exp, AF.Gelu, AF.Copy):
                nc.scalar.activation(warm[:, 1:2], warm[:, 0:1], fn)
            id16 = cst.tile([P, P], bf16)
            make_identity(nc, id16)
            ert2 = cst.tile([P, L], bf16)
            nc.sync.dma_start(ert2[:], ert2_in[:])

            ln1w_bc = ln1b_bc = ln2w_bc = ln2b_bc = None
            if aff1:
                row = cst.tile([1, D], f32, tag="lnrow1a")
                nc.sync.dma_start(row[:], ln1a_in[None, :])
                ln1w_bc = cst.tile([P, D], f32)
                nc.gpsimd.partition_broadcast(ln1w_bc[:], row[:])
                row2 = cst.tile([1, D], f32, tag="lnrow1b")
                nc.sync.dma_start(row2[:], ln1b_in[None, :])
                ln1b_bc = cst.tile([P, D], f32)
                nc.gpsimd.partition_broadcast(ln1b_bc[:], row2[:])
            if aff2:
                row = cst.tile([1, D], f32, tag="lnrow2a")
                nc.sync.dma_start(row[:], ln2a_in[None, :])
                ln2w_bc = cst.tile([P, D], f32)
                nc.gpsimd.partition_broadcast(ln2w_bc[:], row[:])
                row2 = cst.tile([1, D], f32, tag="lnrow2b")
                nc.sync.dma_start(row2[:], ln2b_in[None, :])
                ln2b_bc = cst.tile([P, D], f32)
                nc.gpsimd.partition_broadcast(ln2b_bc[:], row2[:])
            bq_sb = bk_sb = None
            if use_bq:
                bq_sb = cst.tile([P, 4], f32)
                nc.sync.dma_start(bq_sb[:], bq_in[:])
            if use_bk:
                bk_sb = cst.tile([P, 4], f32)
                nc.sync.dma_start(bk_sb[:], bk_in[:])
            bv_bc = None
            if use_bv:
                row = cst.tile([1, NHC * HS], f32, tag="bvrow")
                nc.sync.dma_start(row[:], bv_in[None, :])
                bv_bc = cst.tile([P, NHC * HS], f32)
                nc.gpsimd.partition_broadcast(bv_bc[:], row[:])
            bproj_bc = None
            if use_bproj:
                row = cst.tile([1, D], f32, tag="bprow")
                nc.sync.dma_start(row[:], bproj_in[None, :])
                bproj_bc = cst.tile([P, D], f32)
                nc.gpsimd.partition_broadcast(bproj_bc[:], row[:])
            bfc_sb = None
            if use_bfc:
                bfc_sb = cst.tile([P, FC], f32)
                nc.sync.dma_start(bfc_sb[:], bfc_in[:])
            bfc2_bc = None
            if use_bfc2:
                row = cst.tile([1, D], f32, tag="b2row")
                nc.sync.dma_start(row[:], bfc2_in[None, :])
                bfc2_bc = cst.tile([P, D], f32)
                nc.gpsimd.partition_broadcast(bfc2_bc[:], row[:])

            # Skew DRAM buffers: per (qc, pr-parity): [2 slots][128 rows][srow]
            # bf16.  Write rows at stride srow, read back at stride srow-1 =>
            # row q is realigned by (127 - q); pad cols [wp, wp+128) hold NEG
            # so the causal mask comes back for free.
            negpad = cst.tile([P, 2, P], bf16)
            nc.vector.memset(negpad[:], NEG)
            skewbufs = []
            for qc in range(TC):
                srow = P * (qc + 2)
                slots = []
                for par in range(2):
                    d1 = dram.tile([2 * P * srow], bf16, name=f"skew_{qc}_{par}")
                    slots.append(d1)
                skewbufs.append(slots)

            def emit_negpads():
                for qc in range(TC):
                    srow = P * (qc + 2)
                    wp = P * (qc + 1)
                    for par in range(2):
                        d1 = skewbufs[qc][par]
                        wv_full = d1[:].rearrange(
                            "(s q c) -> q s c", s=2, q=P, c=srow
                        )
                        nc.sync.dma_start(wv_full[:, :, wp : wp + P], negpad[:])

            # ---------------- persistent activation tiles ----------------
            xmyp = es.enter_context(tc.tile_pool(name="xmyp", bufs=1))
            xmy = xmyp.tile([P, T2, D], bf16)
            wfcq_pool = es.enter_context(tc.tile_pool(name="wfcq", bufs=1))
            wqts = [
                wfcq_pool.tile([P, DCH, 1024], bf16, tag=f"wfcq{q % 2}", name=f"wqt{q}")
                for q in range(4)
            ]
            ysb_pool = tc.alloc_tile_pool(name="ysb", bufs=1)
            ysb = ysb_pool.tile([P, 4, L], bf16)

            qkv_pool = tc.alloc_tile_pool(name="qkv", bufs=1)
            qt_sb = [qkv_pool.tile([P, L], bf16, name=f"qt{p}") for p in range(4)]
            kt_sb = [qkv_pool.tile([P, L], bf16, name=f"kt{p}") for p in range(4)]
            # V with a ones column per head: [:, h, 0:64] = V, [:, h, 64] = 1
            v_sb = [qkv_pool.tile([P, NHC, HS + 1], bf16, name=f"v{t}") for t in range(TC)]

            # ---------------- LN1 + transpose + QKV ----------------
            with tc.tile_pool(name="xp", bufs=1) as xph, tc.tile_pool(
                name="hTp", bufs=1
            ) as hTp:
                xs = xph.tile([P, TC, D], bf16)
                for lo, hi in ((0, 1), (1, 2), (2, 4), (4, 6), (6, 8)):
                    nc.sync.dma_start(xs[:, lo:hi, :], x_in[:, lo:hi, :])
                hTT = hTp.tile([P, DCH, L], bf16)
                with tc.tile_pool(name="xh", bufs=1) as xh, tc.tile_pool(
                    name="lnscr", bufs=3
                ) as lnscr:
                    hs = [xh.tile([P, D], bf16, name=f"h{t}") for t in range(TC)]
                    layernorm(
                        tc, nc, (small, lnscr), lambda t: xs[:, t, :], hs, TC,
                        aff1, ln1w_bc, ln1b_bc, eps_t[:],
                    )
                    with tc.tile_pool(name="htps", bufs=3, space="PSUM") as htps:
                        for t in range(TC):
                            tp = htps.tile([P, DCH, P], bf16, tag="htp")
                            for d in range(DCH):
                                nc.tensor.transpose(
                                    tp[:, d, :], hs[t][:, d * P : (d + 1) * P], id16[:]
                                )
                            nc.any.tensor_copy(hTT[:, :, t * P : (t + 1) * P], tp[:])

                # QKV projections (h freed; hTT alive)
                with tc.tile_pool(name="wqkv", bufs=1) as wp_pool, tc.tile_pool(
                    name="qkvps", bufs=5, space="PSUM"
                ) as qps:
                    wq_sb = wp_pool.tile([P, DCH, 512], bf16)
                    wk_sb = wp_pool.tile([P, DCH, 512], bf16)
                    wv_sb = wp_pool.tile([P, DCH, 512], bf16)
                    nc.sync.dma_start(wq_sb[:], wq_in[:])
                    nc.sync.dma_start(wk_sb[:], wk_in[:])
                    nc.sync.dma_start(wv_sb[:], wv_in[:])
                    emit_negpads()
                    # Q^T and K^T: out [128(2 heads), tokens]
                    for p in range(4):
                        for n in range(2):
                            ps = qps.tile([P, 512], f32, tag="qkvp")
                            for d in range(DCH):
                                nc.tensor.matmul(
                                    ps[:],
                                    wq_sb[:, d, p * P : (p + 1) * P],
                                    hTT[:, d, n * 512 : (n + 1) * 512],
                                    start=(d == 0),
                                    stop=(d == DCH - 1),
                                )
                            nc.any.tensor_copy(
                                qt_sb[p][:, n * 512 : (n + 1) * 512], ps[:]
                            )
                            if use_bq:
                                nc.vector.tensor_scalar_add(
                                    qt_sb[p][:, n * 512 : (n + 1) * 512],
                                    qt_sb[p][:, n * 512 : (n + 1) * 512],
                                    bq_sb[:, p : p + 1],
                                )
                        for n in range(2):
                            ps = qps.tile([P, 512], f32, tag="qkvp")
                            for d in range(DCH):
                                nc.tensor.matmul(
                                    ps[:],
                                    wk_sb[:, d, p * P : (p + 1) * P],
                                    hTT[:, d, n * 512 : (n + 1) * 512],
                                    start=(d == 0),
                                    stop=(d == DCH - 1),
                                )
                            nc.any.tensor_copy(
                                kt_sb[p][:, n * 512 : (n + 1) * 512], ps[:]
                            )
                            if use_bk:
                                nc.vector.tensor_scalar_add(
                                    kt_sb[p][:, n * 512 : (n + 1) * 512],
                                    kt_sb[p][:, n * 512 : (n + 1) * 512],
                                    bk_sb[:, p : p + 1],
                                )
                    # V: out [tokens, 512 hs-cols] -> strided into v_sb + ones
                    for t in range(TC):
                        ps = qps.tile([P, 512], f32, tag="qkvp")
                        for d in range(DCH):
                            nc.tensor.matmul(
                                ps[:],
                                hTT[:, d, t * P : (t + 1) * P],
                                wv_sb[:, d, :],
                                start=(d == 0),
                                stop=(d == DCH - 1),
                            )
                        if use_bv:
                            nc.vector.tensor_tensor(
                                ps[:], ps[:], bv_bc[:], op=ALU.add
                            )
                        nc.any.tensor_copy(v_sb[t][:, :, 0:HS], ps[:])
                        nc.vector.memset(v_sb[t][:, :, HS : HS + 1], 1.0)

            # ---------------- attention (transposed logits) ----------------
            nc.scalar.dma_start(xmy[:], xmy_in[:])
            for q in range(2):
                nc.scalar.dma_start(
                    wqts[q][:], wfc_in[:, :, q * 1024 : (q + 1) * 1024]
                )
            with contextlib.ExitStack() as att_es:
                srelp = att_es.enter_context(tc.tile_pool(name="srelp", bufs=2))
                rsbp = att_es.enter_context(tc.tile_pool(name="rsbp", bufs=4))
                attTp = att_es.enter_context(tc.tile_pool(name="attTp", bufs=3))
                y1p = att_es.enter_context(tc.tile_pool(name="y1p", bufs=3))
                nrmp = att_es.enter_context(tc.tile_pool(name="nrmp", bufs=4))
                lps = att_es.enter_context(tc.tile_pool(name="lps", bufs=3, space="PSUM"))
                rps = att_es.enter_context(tc.tile_pool(name="rps", bufs=3, space="PSUM"))
                yps = att_es.enter_context(tc.tile_pool(name="yps", bufs=2, space="PSUM"))

                def emit_rphase(pr):
                    """R = Q Er^T -> DRAM skew write -> skewed read (Srel).
                    Both heads (slots) of the pair in one pass."""
                    srels = []
                    for qc in range(TC):
                        wp = P * (qc + 1)
                        m0 = 896 - P * qc
                        srow = P * (qc + 2)
                        nsub = (wp + 511) // 512
                        d1 = skewbufs[qc][pr % 2]
                        wview = d1[:].rearrange("(s q c) -> q s c", s=2, q=P, c=srow)
                        rsb = rsbp.tile([P, 2, wp], bf16, tag="rsb")
                        for i in range(2):
                            off = i * HS
                            lhsq = qt_sb[pr][off : off + HS, qc * P : (qc + 1) * P]
                            for s in range(nsub):
                                w = min(512, wp - s * 512)
                                rp = rps.tile([P, 512], f32, tag="rp")
                                nc.tensor.matmul(
                                    rp[:, :w],
                                    lhsq,
                                    ert2[off : off + HS, m0 + s * 512 : m0 + s * 512 + w],
                                    start=True,
                                    stop=True,
                                )
                                nc.vector.tensor_copy(
                                    rsb[:, i, s * 512 : s * 512 + w], rp[:, :w]
                                )
                        nc.gpsimd.dma_start(wview[:, :, :wp], rsb[:])
                        srel = srelp.tile([P, 2, wp], bf16, tag=f"srel{qc}")
                        for i in range(2):
                            rv = d1[i * P * srow + 127 : i * P * srow + 127 + P * (srow - 1)]
                            rview = rv.rearrange("(q c) -> q c", c=srow - 1)
                            nc.sync.dma_start(srel[:, i, :], rview[:, :wp])
                        srels.append(srel)
                    return srels

                srel_pending = {0: emit_rphase(0)}
                for pr in range(4):
                    if pr + 1 < 4:
                        srel_pending[pr + 1] = emit_rphase(pr + 1)
                    srels2 = srel_pending.pop(pr)
                    attT2 = [
                        attTp.tile([P, TC, L], bf16, tag="attT", name=f"attT_{pr}_{i}")
                        for i in range(2)
                    ]
                    # logits^T blocks + exp
                    for qc in range(TC):
                        for i in range(2):
                            off = i * HS
                            qmov = qt_sb[pr][off : off + HS, qc * P : (qc + 1) * P]
                            for cg in range(0, qc + 1, 4):
                                ncc = min(4, qc + 1 - cg)
                                lt = lps.tile([P, 4, P], f32, tag="lt")
                                for j in range(ncc):
                                    cc = cg + j
                                    nc.tensor.matmul(
                                        lt[:, j, :],
                                        kt_sb[pr][off : off + HS, cc * P : (cc + 1) * P],
                                        qmov,
                                        start=True,
                                        stop=False,
                                    )
                                    nc.tensor.matmul(
                                        lt[:, j, :],
                                        srels2[qc][:, i, cc * P : (cc + 1) * P],
                                        id16[:],
                                        start=False,
                                        stop=True,
                                    )
                                nc.scalar.activation(
                                    attT2[i][:, cg : cg + ncc, qc * P : (qc + 1) * P],
                                    lt[:, 0:ncc, :],
                                    AF.Exp,
                                )
                    # att @ V with ones-column -> y rows 0..63, denom row 64
                    h0 = 2 * pr
                    for i in range(2):
                        h = h0 + i
                        for n in range(2):
                            yp = yps.tile([P, 512], f32, tag="yp")
                            ccmax = min(TC, 4 * (n + 1))
                            for cc in range(ccmax):
                                lo = max(n * 512, cc * P)
                                w = (n + 1) * 512 - lo
                                nc.tensor.matmul(
                                    yp[0 : HS + 1, lo - n * 512 : lo - n * 512 + w],
                                    v_sb[cc][:, h, :],
                                    attT2[i][:, cc, lo : lo + w],
                                    start=(cc == 0),
                                    stop=(cc == ccmax - 1),
                                )
                            # normalize: rows 0..63 * (1 / row 64)
                            rcp = nrmp.tile([P, 512], f32, tag="rcp")
                            nrm = nrmp.tile([P, 512], f32, tag="nrm")
                            nc.vector.reciprocal(rcp[0:1, :], yp[HS : HS + 1, :])
                            nc.gpsimd.partition_broadcast(nrm[0:HS, :], rcp[0:1, :])
                            if i == 1:
                                nc.vector.tensor_tensor(
                                    ysb[0:HS, pr, n * 512 : (n + 1) * 512],
                                    yp[0:HS, :],
                                    nrm[0:HS, :],
                                    op=ALU.mult,
                                )
                            else:
                                y1 = y1p.tile([P, 512], bf16, tag="y1")
                                nc.vector.tensor_tensor(
                                    y1[0:HS, :],
                                    yp[0:HS, :],
                                    nrm[0:HS, :],
                                    op=ALU.mult,
                                )
                                nc.sync.dma_start(
                                    ysb[HS:P, pr, n * 512 : (n + 1) * 512],
                                    y1[0:HS, :],
                                )

            qkv_pool.release()

            # Internal DRAM for the pairwise ReduceScatter
            cc_in = [dram.tile([L, 512], bf16, name=f"cc_in{n}") for n in range(2)]
            cc_out = [dram.tile([TMY, 512], bf16, name=f"cc_out{n}") for n in range(2)]

            # ---------------- proj (partial) + ReduceScatter ----------------
            with tc.tile_pool(name="wproj", bufs=1) as wpp, tc.tile_pool(
                name="asb", bufs=1
            ) as asbp, tc.tile_pool(name="aps", bufs=4, space="PSUM") as apsp:
                wproj_sb = wpp.tile([P, 4, D], bf16)
                nc.scalar.dma_start(wproj_sb[:], wproj_in[:])
                for n in range(2):
                    asb = asbp.tile([P, TC, 512], bf16, tag=f"asb{n}")
                    for t in range(TC):
                        ap_ = apsp.tile([P, 512], f32, tag="ap")
                        for p in range(4):
                            nc.tensor.matmul(
                                ap_[:],
                                ysb[:, p, t * P : (t + 1) * P],
                                wproj_sb[:, p, n * 512 : (n + 1) * 512],
                                start=(p == 0),
                                stop=(p == 3),
                            )
                        nc.any.tensor_copy(asb[:, t, :], ap_[:])
                        if t % 2 == 1:
                            nc.sync.dma_start(
                                cc_in[n][:].rearrange("(t q) c -> q t c", q=P)[
                                    :, t - 1 : t + 1, :
                                ],
                                asb[:, t - 1 : t + 1, :],
                            )
                    # fire the column-half collective as soon as its inputs
                    # are written; the other half's matmuls overlap it
                    if no_rs:
                        nc.sync.dma_start(cc_out[n][:], cc_in[n][:TMY, :])
                    else:
                        nc.gpsimd.collective_compute(
                            "ReduceScatter",
                            mybir.AluOpType.add,
                            replica_groups=[[0, 1], [2, 3], [4, 5], [6, 7]],
                            ins=[cc_in[n][:]],
                            outs=[cc_out[n][:]],
                        )
            ysb_pool.release()

            # ---------------- residual + LN2 + h2T ----------------
            x2p = es.enter_context(tc.tile_pool(name="x2p", bufs=1))
            x2 = [x2p.tile([P, D], f32, name=f"x2_{t}") for t in range(T2)]
            h2Tp = es.enter_context(tc.tile_pool(name="h2Tp", bufs=1))
            h2TT = h2Tp.tile([P, DCH, TMY], bf16)
            with tc.tile_pool(name="res", bufs=1) as resp, tc.tile_pool(
                name="lnscr2", bufs=2
            ) as lnscr2:
                arb = resp.tile([P, T2, 2, 512], bf16, tag="arb")
                for n in range(2):
                    for g in range(2):
                        nc.sync.dma_start(
                            arb[:, 2 * g : 2 * g + 2, n, :],
                            cc_out[n][:].rearrange("(t q) c -> q t c", q=P)[
                                :, 2 * g : 2 * g + 2, :
                            ],
                        )
                h2 = [resp.tile([P, D], bf16, name=f"h2_{t}") for t in range(T2)]
                for t in range(T2):
                    nc.vector.tensor_tensor(
                        x2[t][:], xmy[:, t, :], arb[:, t, :, :], op=ALU.add
                    )
                    if use_bproj:
                        nc.vector.tensor_tensor(
                            x2[t][:], x2[t][:], bproj_bc[:], op=ALU.add
                        )
                layernorm(
                    tc, nc, (small, lnscr2), lambda t: x2[t][:], h2, T2,
                    aff2, ln2w_bc, ln2b_bc, eps_t[:],
                )
                with tc.tile_pool(name="h2ps", bufs=2, space="PSUM") as h2ps:
                    for t in range(T2):
                        tp = h2ps.tile([P, DCH, P], bf16, tag="h2p")
                        for d in range(DCH):
                            nc.tensor.transpose(
                                tp[:, d, :], h2[t][:, d * P : (d + 1) * P], id16[:]
                            )
                        nc.any.tensor_copy(h2TT[:, :, t * P : (t + 1) * P], tp[:])

            # ---------------- FFN (f-streamed; FFN2 n=0 rides FFN1) ----------
            # FFN1 produces m1T[:, f, :] per f-chunk; FFN2's n=0 column half
            # accumulates in PSUM as each chunk lands, so PE stays dense.  The
            # n=1 half runs as a second f-pass from the kept m1T.
            m1p = es.enter_context(tc.tile_pool(name="m1p", bufs=1))
            m1T = m1p.tile([P, FC, TMY], bf16)
            outp = es.enter_context(tc.tile_pool(name="outp", bufs=1))
            out_sb = outp.tile([P, T2, D], f32)
            with tc.tile_pool(name="wfc2p", bufs=1) as wfc2_pool, tc.tile_pool(
                name="fc1ps", bufs=3, space="PSUM"
            ) as fc1ps, tc.tile_pool(name="fc2ps", bufs=4, space="PSUM") as fc2ps:
                w2n = [
                    wfc2_pool.tile([P, FC, 512], bf16, tag=f"w2n{n}", name=f"w2n{n}")
                    for n in range(2)
                ]
                nc.scalar.dma_start(w2n[0][:], wfc2_in[:, :, 0:512])
                nc.sync.dma_start(w2n[1][:], wfc2_in[:, :, 512:1024])
                pss = [fc2ps.tile([P, 512], f32, tag="fc2", name=f"fc2a_{t}") for t in range(T2)]
                for q in range(4):
                    wq_t = wqts[q]
                    for fl in range(DCH):
                        f = q * DCH + fl
                        mp = fc1ps.tile([P, TMY], f32, tag="m1ps")
                        for d in range(DCH):
                            nc.tensor.matmul(
                                mp[:],
                                wq_t[:, d, fl * P : (fl + 1) * P],
                                h2TT[:, d, :],
                                start=(d == 0),
                                stop=(d == DCH - 1),
                            )
                        if use_bfc:
                            nc.scalar.activation(
                                m1T[:, f, :], mp[:], AF.Gelu,
                                bias=bfc_sb[:, f : f + 1],
                            )
                        else:
                            nc.scalar.activation(m1T[:, f, :], mp[:], AF.Gelu)
                        for t in range(T2):
                            nc.tensor.matmul(
                                pss[t][:],
                                m1T[:, f, t * P : (t + 1) * P],
                                w2n[0][:, f, :],
                                start=(f == 0),
                                stop=(f == FC - 1),
                            )
                    if q + 2 < 4:
                        nc.scalar.dma_start(
                            wqts[q + 2][:],
                            wfc_in[:, :, (q + 2) * 1024 : (q + 3) * 1024],
                        )
                for t in range(T2):
                    nc.vector.tensor_tensor(
                        out_sb[:, t, 0:512],
                        pss[t][:],
                        x2[t][:, 0:512],
                        op=ALU.add,
                    )
                # second pass: n=1 column half, t-major so the tail pipelines
                outv = out_dram[:].rearrange("(t q) c -> q t c", q=P)
                for t in range(T2):
                    ps2 = fc2ps.tile([P, 512], f32, tag="fc2", name=f"fc2b_{t}")
                    for f in range(FC):
                        nc.tensor.matmul(
                            ps2[:],
                            m1T[:, f, t * P : (t + 1) * P],
                            w2n[1][:, f, :],
                            start=(f == 0),
                            stop=(f == FC - 1),
                        )
                    nc.vector.tensor_tensor(
                        out_sb[:, t, 512:1024],
                        ps2[:],
                        x2[t][:, 512:1024],
                        op=ALU.add,
                    )
                    if use_bfc2:
                        nc.vector.tensor_tensor(
                            out_sb[:, t, :], out_sb[:, t, :], bfc2_bc[:],
                            op=ALU.add,
                        )
                    nc.sync.dma_start(outv[:, t, :], out_sb[:, t, :])

    nc.compile()
    return nc


def _get_program(flags):
    if flags not in _PROGRAM_CACHE:
        _PROGRAM_CACHE[flags] = _build_program(flags)
    return _PROGRAM_CACHE[flags]


def kernel(
    x,
    ln1_w,
    ln1_b,
    Wqkv,
    bqkv,
    Wproj,
    bproj,
    Er,
    ln2_w,
    ln2_b,
    Wfc,
    bfc,
    Wfc2,
    bfc2,
):
    import ml_dtypes
    from concourse.bass_utils import run_bass_kernel_spmd

    bf = ml_dtypes.bfloat16
    x = np.asarray(x, np.float32)
    f = np.float32
    ntriv = lambda a, v: not np.all(np.asarray(a) == v)
    flags = (
        ntriv(ln1_w, 1) or ntriv(ln1_b, 0),
        ntriv(ln2_w, 1) or ntriv(ln2_b, 0),
        ntriv(bqkv[:D], 0),
        ntriv(bqkv[D : 2 * D], 0),
        ntriv(bqkv[2 * D :], 0),
        ntriv(bproj, 0),
        ntriv(bfc, 0),
        ntriv(bfc2, 0),
    )
    nc = _get_program(flags)

    c = np.ascontiguousarray

    def pack_w(m, nch):
        # [rows, cols] -> [128, nch, cols] where rows = nch*128 chunk-major
        m = np.asarray(m, f)
        rows, cols = m.shape
        return c(m.reshape(nch, P, cols).transpose(1, 0, 2).astype(bf))

    ert2_f = np.concatenate([np.asarray(Er, f).T, np.asarray(Er, f).T], axis=0)
    ert2_pk = c(ert2_f.astype(bf))
    wfc_pk = pack_w(np.asarray(Wfc), DCH)
    wfc2_pk = pack_w(np.asarray(Wfc2), FC)

    in_maps = []
    for core in range(8):
        b, half = divmod(core, 2)
        hs0, hs1 = half * 512, (half + 1) * 512
        bq = np.asarray(bqkv[:D][hs0:hs1], f) * SCALE
        bk = np.asarray(bqkv[D : 2 * D][hs0:hs1], f)
        wq = np.asarray(Wqkv)[:, 0:D][:, hs0:hs1] * SCALE
        x_r = x[b].reshape(TC, P, D)
        x_pk = c(x_r.transpose(1, 0, 2).astype(bf))
        xmy_pk = c(x_r[half * T2 : (half + 1) * T2].transpose(1, 0, 2).astype(bf))
        in_maps.append(
            {
                "x": x_pk,
                "x_my": xmy_pk,
                "wq": pack_w(wq, DCH),
                "wk": pack_w(np.asarray(Wqkv)[:, D : 2 * D][:, hs0:hs1], DCH),
                "wv": pack_w(np.asarray(Wqkv)[:, 2 * D :][:, hs0:hs1], DCH),
                "wproj": pack_w(np.asarray(Wproj)[hs0:hs1, :].reshape(4, 2, HS, D)[:, ::-1].reshape(512, D), 4),
                "ert2": ert2_pk,
                "wfc": wfc_pk,
                "wfc2": wfc2_pk,
                "ln1a": c(np.asarray(ln1_w), f),
                "ln1b": c(np.asarray(ln1_b), f),
                "ln2a": c(np.asarray(ln2_w), f),
                "ln2b": c(np.asarray(ln2_b), f),
                "bq": c(bq.reshape(4, P).T, f),
                "bk": c(bk.reshape(4, P).T, f),
                "bv": c(np.asarray(bqkv[2 * D :][hs0:hs1]), f),
                "bproj": c(np.asarray(bproj), f),
                "bfc": c(np.asarray(bfc).reshape(FC, P).T, f),
                "bfc2": c(np.asarray(bfc2), f),
            }
        )

    trace = bool(int(os.environ.get("KERNEL_TRACE", "0")))
    res = run_bass_kernel_spmd(nc, in_maps, list(range(8)), trace=trace)
    global LAST_EXEC_NS, LAST_RESULT
    LAST_EXEC_NS = res.exec_time_ns
    LAST_RESULT = res
    out = np.empty((B, L, D), np.float32)
    for core in range(8):
        b, half = divmod(core, 2)
        out[b, half * 512 : (half + 1) * 512] = res.results[core]["out_my"]
    return out


LAST_EXEC_NS = None
LAST_RESULT = None


# revision 93
# speedup vs baseline: 1.0147x; 1.0063x over previous
"""Trainium2 Bass kernel for nn_BlockWithCache (Music-Transformer block w/ rel-pos).

Sharding (8 NeuronCores, uniform SPMD program; per-core differences live in the
input data only):
  - core c: batch element b = c//2, tensor-parallel half = c%2.
  - Attention: TP over heads — each core computes its 8 of 16 heads for the
    full 1024-token sequence (weight column slices supplied by the host).
  - Wproj row-slices produce partial attention outputs; a pairwise
    ReduceScatter(add) both completes the sum and splits tokens in half.
  - From the residual on: token-split — each core owns 512 tokens through
    LN2 + FFN (full 4*D hidden) and writes a disjoint output half.

v2 notes:
  - bf16 everywhere on the matmul path; weights host-packed into SBUF tile
    layouts so each matrix loads with 1-3 large contiguous DMAs.
  - Transposed attention: logits are computed as [key, query] (kt stationary,
    qt moving); Srel (read back from the DRAM skew buffer in [q, k] rows) is
    accumulated into the same PSUM block by a matmul with Srel as the
    stationary operand, which transposes it for free.  exp() then writes
    attT directly - no PE transposes and no separate PSUM->SBUF copies.
  - Softmax denominators come from a ones-column appended to V (attV PSUM row
    64); normalization is folded into the yp->ysb copy as a broadcast mult.
  - Skew/negpad DMAs issue from the (otherwise idle) Pool engine, bypassing
    the shared HWDGE descriptor-generation bottleneck.
  - FFN runs in two 256-token halves so FFN2(half A) overlaps FFN1(half B).
"""

import os
import sys

os.environ.setdefault("MYCRO_LOCAL_CACHE", "1")
if "/opt/trn_rl_repo" not in sys.path:
    sys.path.insert(0, "/opt/trn_rl_repo")

import numpy as np

B, L, D, H = 4, 1024, 1024, 16
HS = D // H          # 64
P = 128
TC = L // P          # 8 token chunks
DCH = D // P         # 8 feature chunks
NHC = H // 2         # 8 heads per core
FD = 4 * D           # 4096
FC = FD // P         # 32
TMY = L // 2         # 512 tokens owned after RS
T2 = TMY // P        # 4
EPS = 1e-5
SCALE = 1.0 / 8.0    # 1/sqrt(HS)
NEG = -1.0e9

_PROGRAM_CACHE = {}


def _build_program(flags, no_rs=False):
    import concourse.mybir as mybir
    import concourse.tile as tile
    from concourse import bacc
    from concourse.masks import make_identity

    (aff1, aff2, use_bq, use_bk, use_bv, use_bproj, use_bfc, use_bfc2) = flags

    f32 = mybir.dt.float32
    bf16 = mybir.dt.bfloat16
    fp8 = mybir.dt.float8e4
    AF = mybir.ActivationFunctionType
    ALU = mybir.AluOpType
    AX = mybir.AxisListType

    nc = bacc.Bacc("TRN2", target_bir_lowering=False, debug=False, num_devices=8)

    # Host-packed parameters (already in SBUF tile layout; see kernel()).
    x_in = nc.declare_dram_parameter("x", [P, TC, D], bf16, isOutput=False)
    xmy_in = nc.declare_dram_parameter("x_my", [P, T2, D], bf16, isOutput=False)
    wq_in = nc.declare_dram_parameter("wq", [P, DCH, 512], bf16, isOutput=False)
    wk_in = nc.declare_dram_parameter("wk", [P, DCH, 512], bf16, isOutput=False)
    wv_in = nc.declare_dram_parameter("wv", [P, DCH, 512], bf16, isOutput=False)
    wproj_in = nc.declare_dram_parameter("wproj", [P, 4, D], bf16, isOutput=False)
    ert2_in = nc.declare_dram_parameter("ert2", [P, L], bf16, isOutput=False)
    wfc_in = nc.declare_dram_parameter("wfc", [P, DCH, FD], bf16, isOutput=False)
    wfc2_in = nc.declare_dram_parameter("wfc2", [P, FC, D], bf16, isOutput=False)
    # Always-declared small params (cheap; used only when flags set)
    ln1a_in = nc.declare_dram_parameter("ln1a", [D], f32, isOutput=False)
    ln1b_in = nc.declare_dram_parameter("ln1b", [D], f32, isOutput=False)
    ln2a_in = nc.declare_dram_parameter("ln2a", [D], f32, isOutput=False)
    ln2b_in = nc.declare_dram_parameter("ln2b", [D], f32, isOutput=False)
    bq_in = nc.declare_dram_parameter("bq", [P, 4], f32, isOutput=False)
    bk_in = nc.declare_dram_parameter("bk", [P, 4], f32, isOutput=False)
    bv_in = nc.declare_dram_parameter("bv", [NHC * HS], f32, isOutput=False)
    bproj_in = nc.declare_dram_parameter("bproj", [D], f32, isOutput=False)
    bfc_in = nc.declare_dram_parameter("bfc", [P, FC], f32, isOutput=False)
    bfc2_in = nc.declare_dram_parameter("bfc2", [D], f32, isOutput=False)

    out_dram = nc.declare_dram_parameter("out_my", [TMY, D], f32, isOutput=True)

    def layernorm(tc, nc, pools, xin, hs, nchunks, aff, wbc, bbc, eps_ap,
                  sts=None):
        """Per-chunk two-pass LN; xin(t) returns the [P, D] f32 input AP,
        hs[t] is the bf16 output tile.  If sts is given, sts[t] is a [P, 8]
        stats tile whose col 0 already holds the row sums."""
        small, scratch = pools
        for t in range(nchunks):
            xap = xin(t)
            if sts is not None:
                st = sts[t]
            else:
                st = small.tile([P, 8], f32, tag="ln_st")
                # st cols: 0 sum, 1 sumsq, 2 mu, 3 mu2, 4 var, 5 std, 6 rstd, 7 mur
                nc.vector.reduce_sum(st[:, 0:1], xap, axis=AX.X)
            sq = scratch.tile([P, D], f32, tag="ln_sq")
            nc.scalar.activation(sq[:], xap, AF.Square, accum_out=st[:, 1:2])
            nc.vector.tensor_scalar_mul(st[:, 2:3], st[:, 0:1], 1.0 / D)
            nc.vector.tensor_tensor(st[:, 3:4], st[:, 2:3], st[:, 2:3], op=ALU.mult)
            nc.vector.tensor_scalar(
                st[:, 4:5], st[:, 1:2], 1.0 / D, st[:, 3:4],
                op0=ALU.mult, op1=ALU.subtract,
            )
            nc.scalar.activation(st[:, 5:6], st[:, 4:5], AF.Sqrt, bias=eps_ap)
            nc.vector.reciprocal(st[:, 6:7], st[:, 5:6])
            nc.vector.tensor_tensor(st[:, 7:8], st[:, 2:3], st[:, 6:7], op=ALU.mult)
            nc.vector.tensor_scalar(
                hs[t][:],
                xap,
                st[:, 6:7],
                st[:, 7:8],
                op0=ALU.mult,
                op1=ALU.subtract,
            )
            if aff:
                nc.vector.tensor_tensor(hs[t][:], hs[t][:], wbc[:], op=ALU.mult)
                nc.vector.tensor_tensor(hs[t][:], hs[t][:], bbc[:], op=ALU.add)

    with tile.TileContext(nc) as tc:
        import contextlib

        with contextlib.ExitStack() as es:
            cst = es.enter_context(tc.tile_pool(name="cst", bufs=1))
            small = es.enter_context(tc.tile_pool(name="small", bufs=2))
            dram = es.enter_context(tc.tile_pool(name="dram", bufs=1, space="DRAM"))

            eps_t = cst.tile([P, 1], f32)
            nc.vector.memset(eps_t[:], EPS)
            warm = cst.tile([P, 2], f32)
            nc.vector.memset(warm[:], 1.0)
            for fn in (AF.Square, AF.Sqrt, AF.Exp, AF.Gelu, AF.Copy):
                nc.scalar.activation(warm[:, 1:2], warm[:, 0:1], fn)
            id16 = cst.tile([P, P], bf16)
            make_identity(nc, id16)
            ert2 = cst.tile([P, L], bf16)
            nc.sync.dma_start(ert2[:], ert2_in[:])

            ln1w_bc = ln1b_bc = ln2w_bc = ln2b_bc = None
            if aff1:
                row = cst.tile([1, D], f32, tag="lnrow1a")
                nc.sync.dma_start(row[:], ln1a_in[None, :])
                ln1w_bc = cst.tile([P, D], f32)
                nc.gpsimd.partition_broadcast(ln1w_bc[:], row[:])
                row2 = cst.tile([1, D], f32, tag="lnrow1b")
                nc.sync.dma_start(row2[:], ln1b_in[None, :])
                ln1b_bc = cst.tile([P, D], f32)
                nc.gpsimd.partition_broadcast(ln1b_bc[:], row2[:])
            if aff2:
                row = cst.tile([1, D], f32, tag="lnrow2a")
                nc.sync.dma_start(row[:], ln2a_in[None, :])
                ln2w_bc = cst.tile([P, D], f32)
                nc.gpsimd.partition_broadcast(ln2w_bc[:], row[:])
                row2 = cst.tile([1, D], f32, tag="lnrow2b")
                nc.sync.dma_start(row2[:], ln2b_in[None, :])
                ln2b_bc = cst.tile([P, D], f32)
                nc.gpsimd.partition_broadcast(ln2b_bc[:], row2[:])
            bq_sb = bk_sb = None
            if use_bq:
                bq_sb = cst.tile([P, 4], f32)
                nc.sync.dma_start(bq_sb[:], bq_in[:])
            if use_bk:
                bk_sb = cst.tile([P, 4], f32)
                nc.sync.dma_start(bk_sb[:], bk_in[:])
            bv_bc = None
            if use_bv:
                row = cst.tile([1, NHC * HS], f32, tag="bvrow")
                nc.sync.dma_start(row[:], bv_in[None, :])
                bv_bc = cst.tile([P, NHC * HS], f32)
                nc.gpsimd.partition_broadcast(bv_bc[:], row[:])
            bproj_bc = None
            if use_bproj:
                row = cst.tile([1, D], f32, tag="bprow")
                nc.sync.dma_start(row[:], bproj_in[None, :])
                bproj_bc = cst.tile([P, D], f32)
                nc.gpsimd.partition_broadcast(bproj_bc[:], row[:])
            bfc_sb = None
            if use_bfc:
                bfc_sb = cst.tile([P, FC], f32)
                nc.sync.dma_start(bfc_sb[:], bfc_in[:])
            bfc2_bc = None
            if use_bfc2:
                row = cst.tile([1, D], f32, tag="b2row")
                nc.sync.dma_start(row[:], bfc2_in[None, :])
                bfc2_bc = cst.tile([P, D], f32)
                nc.gpsimd.partition_broadcast(bfc2_bc[:], row[:])

            # Skew DRAM buffers: per (qc, pr-parity): [2 slots][128 rows][srow]
            # bf16.  Write rows at stride srow, read back at stride srow-1 =>
            # row q is realigned by (127 - q); pad cols [wp, wp+128) hold NEG
            # so the causal mask comes back for free.
            negpad = cst.tile([P, 2, P], bf16)
            nc.vector.memset(negpad[:], NEG)
            skewbufs = []
            for qc in range(TC):
                srow = P * (qc + 2)
                slots = []
                for par in range(2):
                    d1 = dram.tile([2 * P * srow], bf16, name=f"skew_{qc}_{par}")
                    slots.append(d1)
                skewbufs.append(slots)

            def emit_negpads():
                for qc in range(TC):
                    srow = P * (qc + 2)
                    wp = P * (qc + 1)
                    for par in range(2):
                        d1 = skewbufs[qc][par]
                        wv_full = d1[:].rearrange(
                            "(s q c) -> q s c", s=2, q=P, c=srow
                        )
                        nc.sync.dma_start(wv_full[:, :, wp : wp + P], negpad[:])

            # ---------------- persistent activation tiles ----------------
            xmyp = es.enter_context(tc.tile_pool(name="xmyp", bufs=1))
            xmy = xmyp.tile([P, T2, D], bf16)
            wfcq_pool = es.enter_context(tc.tile_pool(name="wfcq", bufs=1))
            wqts = [
                wfcq_pool.tile([P, DCH, 1024], bf16, tag=f"wfcq{q % 2}", name=f"wqt{q}")
                for q in range(4)
            ]
            ysb_pool = tc.alloc_tile_pool(name="ysb", bufs=1)
            ysb = ysb_pool.tile([P, 4, L], bf16)

            qkv_pool = tc.alloc_tile_pool(name="qkv", bufs=1)
            qt_sb = [qkv_pool.tile([P, L], bf16, name=f"qt{p}") for p in range(4)]
            kt_sb = [qkv_pool.tile([P, L], bf16, name=f"kt{p}") for p in range(4)]
            # V with a ones column per head: [:, h, 0:64] = V, [:, h, 64] = 1
            v_sb = [qkv_pool.tile([P, NHC, HS + 1], bf16, name=f"v{t}") for t in range(TC)]

            # ---------------- LN1 + transpose + QKV ----------------
            with tc.tile_pool(name="xp", bufs=1) as xph, tc.tile_pool(
                name="hTp", bufs=1
            ) as hTp:
                xs = xph.tile([P, TC, D], bf16)
                for lo, hi in ((0, 1), (1, 2), (2, 4), (4, 6), (6, 8)):
                    nc.sync.dma_start(xs[:, lo:hi, :], x_in[:, lo:hi, :])
                hTT = hTp.tile([P, DCH, L], bf16)
                with tc.tile_pool(name="xh", bufs=1) as xh, tc.tile_pool(
                    name="lnscr", bufs=3
                ) as lnscr:
                    hs = [xh.tile([P, D], bf16, name=f"h{t}") for t in range(TC)]
                    layernorm(
                        tc, nc, (small, lnscr), lambda t: xs[:, t, :], hs, TC,
                        aff1, ln1w_bc, ln1b_bc, eps_t[:],
                    )
                    with tc.tile_pool(name="htps", bufs=3, space="PSUM") as htps:
                        for t in range(TC):
                            tp = htps.tile([P, DCH, P], bf16, tag="htp")
                            for d in range(DCH):
                                nc.tensor.transpose(
                                    tp[:, d, :], hs[t][:, d * P : (d + 1) * P], id16[:]
                                )
                            nc.any.tensor_copy(hTT[:, :, t * P : (t + 1) * P], tp[:])

                # QKV projections (h freed; hTT alive)
                with tc.tile_pool(name="wqkv", bufs=1) as wp_pool, tc.tile_pool(
                    name="qkvps", bufs=5, space="PSUM"
                ) as qps:
                    wq_sb = wp_pool.tile([P, DCH, 512], bf16)
                    wk_sb = wp_pool.tile([P, DCH, 512], bf16)
                    wv_sb = wp_pool.tile([P, DCH, 512], bf16)
                    nc.sync.dma_start(wq_sb[:], wq_in[:])
                    nc.sync.dma_start(wk_sb[:], wk_in[:])
                    nc.sync.dma_start(wv_sb[:], wv_in[:])
                    emit_negpads()
                    # Q^T and K^T: out [128(2 heads), tokens]
                    for p in range(4):
                        for n in range(2):
                            ps = qps.tile([P, 512], f32, tag="qkvp")
                            for d in range(DCH):
                                nc.tensor.matmul(
                                    ps[:],
                                    wq_sb[:, d, p * P : (p + 1) * P],
                                    hTT[:, d, n * 512 : (n + 1) * 512],
                                    start=(d == 0),
                                    stop=(d == DCH - 1),
                                )
                            nc.any.tensor_copy(
                                qt_sb[p][:, n * 512 : (n + 1) * 512], ps[:]
                            )
                            if use_bq:
                                nc.vector.tensor_scalar_add(
                                    qt_sb[p][:, n * 512 : (n + 1) * 512],
                                    qt_sb[p][:, n * 512 : (n + 1) * 512],
                                    bq_sb[:, p : p + 1],
                                )
                        for n in range(2):
                            ps = qps.tile([P, 512], f32, tag="qkvp")
                            for d in range(DCH):
                                nc.tensor.matmul(
                                    ps[:],
                                    wk_sb[:, d, p * P : (p + 1) * P],
                                    hTT[:, d, n * 512 : (n + 1) * 512],
                                    start=(d == 0),
                                    stop=(d == DCH - 1),
                                )
                            nc.any.tensor_copy(
                                kt_sb[p][:, n * 512 : (n + 1) * 512], ps[:]
                            )
                            if use_bk:
                                nc.vector.tensor_scalar_add(
                                    kt_sb[p][:, n * 512 : (n + 1) * 512],
                                    kt_sb[p][:, n * 512 : (n + 1) * 512],
                                    bk_sb[:, p : p + 1],
                                )
                    # V: out [tokens, 512 hs-cols] -> strided into v_sb + ones
                    for t in range(TC):
                        ps = qps.tile([P, 512], f32, tag="qkvp")
                        for d in range(DCH):
                            nc.tensor.matmul(
                                ps[:],
                                hTT[:, d, t * P : (t + 1) * P],
                                wv_sb[:, d, :],
                                start=(d == 0),
                                stop=(d == DCH - 1),
                            )
                        if use_bv:
                            nc.vector.tensor_tensor(
                                ps[:], ps[:], bv_bc[:], op=ALU.add
                            )
                        nc.any.tensor_copy(v_sb[t][:, :, 0:HS], ps[:])
                        nc.vector.memset(v_sb[t][:, :, HS : HS + 1], 1.0)

            # ---------------- attention (transposed logits) ----------------
            nc.scalar.dma_start(xmy[:], xmy_in[:])
            for q in range(2):
                nc.scalar.dma_start(
                    wqts[q][:], wfc_in[:, :, q * 1024 : (q + 1) * 1024]
                )
            with contextlib.ExitStack() as att_es:
                srelp = att_es.enter_context(tc.tile_pool(name="srelp", bufs=2))
                rsbp = att_es.enter_context(tc.tile_pool(name="rsbp", bufs=4))
                attTp = att_es.enter_context(tc.tile_pool(name="attTp", bufs=3))
                y1p = att_es.enter_context(tc.tile_pool(name="y1p", bufs=3))
                nrmp = att_es.enter_context(tc.tile_pool(name="nrmp", bufs=4))
                lps = att_es.enter_context(tc.tile_pool(name="lps", bufs=3, space="PSUM"))
                rps = att_es.enter_context(tc.tile_pool(name="rps", bufs=3, space="PSUM"))
                yps = att_es.enter_context(tc.tile_pool(name="yps", bufs=2, space="PSUM"))

                def emit_rphase(pr):
                    """R = Q Er^T -> DRAM skew write -> skewed read (Srel).
                    Both heads (slots) of the pair in one pass."""
                    srels = []
                    for qc in range(TC):
                        wp = P * (qc + 1)
                        m0 = 896 - P * qc
                        srow = P * (qc + 2)
                        nsub = (wp + 511) // 512
                        d1 = skewbufs[qc][pr % 2]
                        wview = d1[:].rearrange("(s q c) -> q s c", s=2, q=P, c=srow)
                        rsb = rsbp.tile([P, 2, wp], bf16, tag="rsb")
                        for i in range(2):
                            off = i * HS
                            lhsq = qt_sb[pr][off : off + HS, qc * P : (qc + 1) * P]
                            for s in range(nsub):
                                w = min(512, wp - s * 512)
                                rp = rps.tile([P, 512], f32, tag="rp")
                                nc.tensor.matmul(
                                    rp[:, :w],
                                    lhsq,
                                    ert2[off : off + HS, m0 + s * 512 : m0 + s * 512 + w],
                                    start=True,
                                    stop=True,
                                )
                                nc.vector.tensor_copy(
                                    rsb[:, i, s * 512 : s * 512 + w], rp[:, :w]
                                )
                        nc.gpsimd.dma_start(wview[:, :, :wp], rsb[:])
                        srel = srelp.tile([P, 2, wp], bf16, tag=f"srel{qc}")
                        for i in range(2):
                            rv = d1[i * P * srow + 127 : i * P * srow + 127 + P * (srow - 1)]
                            rview = rv.rearrange("(q c) -> q c", c=srow - 1)
                            nc.sync.dma_start(srel[:, i, :], rview[:, :wp])
                        srels.append(srel)
                    return srels

                srel_pending = {0: emit_rphase(0)}
                for pr in range(4):
                    if pr + 1 < 4:
                        srel_pending[pr + 1] = emit_rphase(pr + 1)
                    srels2 = srel_pending.pop(pr)
                    attT2 = [
                        attTp.tile([P, TC, L], bf16, tag="attT", name=f"attT_{pr}_{i}")
                        for i in range(2)
                    ]
                    # logits^T blocks + exp
                    for qc in range(TC):
                        for i in range(2):
                            off = i * HS
                            qmov = qt_sb[pr][off : off + HS, qc * P : (qc + 1) * P]
                            for cg in range(0, qc + 1, 4):
                                ncc = min(4, qc + 1 - cg)
                                lt = lps.tile([P, 4, P], f32, tag="lt")
                                for j in range(ncc):
                                    cc = cg + j
                                    nc.tensor.matmul(
                                        lt[:, j, :],
                                        kt_sb[pr][off : off + HS, cc * P : (cc + 1) * P],
                                        qmov,
                                        start=True,
                                        stop=False,
                                    )
                                    nc.tensor.matmul(
                                        lt[:, j, :],
                                        srels2[qc][:, i, cc * P : (cc + 1) * P],
                                        id16[:],
                                        start=False,
                                        stop=True,
                                    )
                                nc.scalar.activation(
                                    attT2[i][:, cg : cg + ncc, qc * P : (qc + 1) * P],
                                    lt[:, 0:ncc, :],
                                    AF.Exp,
                                )
                    # att @ V with ones-column -> y rows 0..63, denom row 64
                    h0 = 2 * pr
                    for i in range(2):
                        h = h0 + i
                        for n in range(2):
                            yp = yps.tile([P, 512], f32, tag="yp")
                            ccmax = min(TC, 4 * (n + 1))
                            for cc in range(ccmax):
                                lo = max(n * 512, cc * P)
                                w = (n + 1) * 512 - lo
                                nc.tensor.matmul(
                                    yp[0 : HS + 1, lo - n * 512 : lo - n * 512 + w],
                                    v_sb[cc][:, h, :],
                                    attT2[i][:, cc, lo : lo + w],
                                    start=(cc == 0),
                                    stop=(cc == ccmax - 1),
                                )
                            # normalize: rows 0..63 * (1 / row 64)
                            rcp = nrmp.tile([P, 512], f32, tag="rcp")
                            nrm = nrmp.tile([P, 512], f32, tag="nrm")
                            nc.vector.reciprocal(rcp[0:1, :], yp[HS : HS + 1, :])
                            nc.gpsimd.partition_broadcast(nrm[0:HS, :], rcp[0:1, :])
                            if i == 1:
                                nc.vector.tensor_tensor(
                                    ysb[0:HS, pr, n * 512 : (n + 1) * 512],
                                    yp[0:HS, :],
                                    nrm[0:HS, :],
                                    op=ALU.mult,
                                )
                            else:
                                y1 = y1p.tile([P, 512], bf16, tag="y1")
                                nc.vector.tensor_tensor(
                                    y1[0:HS, :],
                                    yp[0:HS, :],
                                    nrm[0:HS, :],
                                    op=ALU.mult,
                                )
                                nc.sync.dma_start(
                                    ysb[HS:P, pr, n * 512 : (n + 1) * 512],
                                    y1[0:HS, :],
                                )

            qkv_pool.release()

            # Internal DRAM for the pairwise ReduceScatter
            cc_in = [dram.tile([L, 512], bf16, name=f"cc_in{n}") for n in range(2)]
            cc_out = [dram.tile([TMY, 512], bf16, name=f"cc_out{n}") for n in range(2)]

            # ---------------- proj (partial) + ReduceScatter ----------------
            with tc.tile_pool(name="wproj", bufs=1) as wpp, tc.tile_pool(
                name="asb", bufs=1
            ) as asbp, tc.tile_pool(name="aps", bufs=4, space="PSUM") as apsp:
                wproj_sb = wpp.tile([P, 4, D], bf16)
                nc.scalar.dma_start(wproj_sb[:], wproj_in[:])
                for n in range(2):
                    asb = asbp.tile([P, TC, 512], bf16, tag=f"asb{n}")
                    for t in range(TC):
                        ap_ = apsp.tile([P, 512], f32, tag="ap")
                        for p in range(4):
                            nc.tensor.matmul(
                                ap_[:],
                                ysb[:, p, t * P : (t + 1) * P],
                                wproj_sb[:, p, n * 512 : (n + 1) * 512],
                                start=(p == 0),
                                stop=(p == 3),
                            )
                        nc.any.tensor_copy(asb[:, t, :], ap_[:])
                        if t % 2 == 1:
                            nc.sync.dma_start(
                                cc_in[n][:].rearrange("(t q) c -> q t c", q=P)[
                                    :, t - 1 : t + 1, :
                                ],
                                asb[:, t - 1 : t + 1, :],
                            )
                    # fire the column-half collective as soon as its inputs
                    # are written; the other half's matmuls overlap it
                    if no_rs:
                        nc.sync.dma_start(cc_out[n][:], cc_in[n][:TMY, :])
                    else:
                        nc.gpsimd.collective_compute(
                            "ReduceScatter",
                            mybir.AluOpType.add,
                            replica_groups=[[0, 1], [2, 3], [4, 5], [6, 7]],
                            ins=[cc_in[n][:]],
                            outs=[cc_out[n][:]],
                        )
            ysb_pool.release()

            # ---------------- residual + LN2 + h2T ----------------
            x2p = es.enter_context(tc.tile_pool(name="x2p", bufs=1))
            x2 = [x2p.tile([P, D], f32, name=f"x2_{t}") for t in range(T2)]
            h2Tp = es.enter_context(tc.tile_pool(name="h2Tp", bufs=1))
            h2TT = h2Tp.tile([P, DCH, TMY], bf16)
            with tc.tile_pool(name="res", bufs=1) as resp, tc.tile_pool(
                name="lnscr2", bufs=2
            ) as lnscr2:
                arb = resp.tile([P, T2, 2, 512], bf16, tag="arb")
                for n in range(2):
                    for g in range(2):
                        nc.sync.dma_start(
                            arb[:, 2 * g : 2 * g + 2, n, :],
                            cc_out[n][:].rearrange("(t q) c -> q t c", q=P)[
                                :, 2 * g : 2 * g + 2, :
                            ],
                        )
                h2 = [resp.tile([P, D], bf16, name=f"h2_{t}") for t in range(T2)]
                sts2 = [resp.tile([P, 8], f32, name=f"st2_{t}") for t in range(T2)]
                for t in range(T2):
                    if use_bproj:
                        nc.vector.tensor_tensor(
                            x2[t][:], xmy[:, t, :], arb[:, t, :, :], op=ALU.add
                        )
                        nc.vector.scalar_tensor_tensor(
                            x2[t][:], x2[t][:], 1.0, bproj_bc[:],
                            op0=ALU.mult, op1=ALU.add,
                            accum_out=sts2[t][:, 0:1],
                        )
                    else:
                        nc.vector.scalar_tensor_tensor(
                            x2[t][:], xmy[:, t, :], 1.0, arb[:, t, :, :],
                            op0=ALU.mult, op1=ALU.add,
                            accum_out=sts2[t][:, 0:1],
                        )
                layernorm(
                    tc, nc, (small, lnscr2), lambda t: x2[t][:], h2, T2,
                    aff2, ln2w_bc, ln2b_bc, eps_t[:], sts=sts2,
                )
                with tc.tile_pool(name="h2ps", bufs=2, space="PSUM") as h2ps:
                    for t in range(T2):
                        tp = h2ps.tile([P, DCH, P], bf16, tag="h2p")
                        for d in range(DCH):
                            nc.tensor.transpose(
                                tp[:, d, :], h2[t][:, d * P : (d + 1) * P], id16[:]
                            )
                        nc.any.tensor_copy(h2TT[:, :, t * P : (t + 1) * P], tp[:])

            # ---------------- FFN (f-streamed; FFN2 n=0 rides FFN1) ----------
            # FFN1 produces m1T[:, f, :] per f-chunk; FFN2's n=0 column half
            # accumulates in PSUM as each chunk lands, so PE stays dense.  The
            # n=1 half runs as a second f-pass from the kept m1T.
            m1p = es.enter_context(tc.tile_pool(name="m1p", bufs=1))
            m1T = m1p.tile([P, FC, TMY], bf16)
            outp = es.enter_context(tc.tile_pool(name="outp", bufs=1))
            out_sb = outp.tile([P, T2, D], f32)
            with tc.tile_pool(name="wfc2p", bufs=1) as wfc2_pool, tc.tile_pool(
                name="fc1ps", bufs=3, space="PSUM"
            ) as fc1ps, tc.tile_pool(name="fc2ps", bufs=4, space="PSUM") as fc2ps:
                w2n = [
                    wfc2_pool.tile([P, FC, 512], bf16, tag=f"w2n{n}", name=f"w2n{n}")
                    for n in range(2)
                ]
                nc.scalar.dma_start(w2n[0][:], wfc2_in[:, :, 0:512])
                nc.sync.dma_start(w2n[1][:], wfc2_in[:, :, 512:1024])
                pss = [fc2ps.tile([P, 512], f32, tag="fc2", name=f"fc2a_{t}") for t in range(T2)]
                for q in range(4):
                    wq_t = wqts[q]
                    for fl in range(DCH):
                        f = q * DCH + fl
                        mp = fc1ps.tile([P, TMY], f32, tag="m1ps")
                        for d in range(DCH):
                            nc.tensor.matmul(
                                mp[:],
                                wq_t[:, d, fl * P : (fl + 1) * P],
                                h2TT[:, d, :],
                                start=(d == 0),
                                stop=(d == DCH - 1),
                            )
                        if use_bfc:
                            nc.scalar.activation(
                                m1T[:, f, :], mp[:], AF.Gelu,
                                bias=bfc_sb[:, f : f + 1],
                            )
                        else:
                            nc.scalar.activation(m1T[:, f, :], mp[:], AF.Gelu)
                        for t in range(T2):
                            nc.tensor.matmul(
                                pss[t][:],
                                m1T[:, f, t * P : (t + 1) * P],
                                w2n[0][:, f, :],
                                start=(f == 0),
                                stop=(f == FC - 1),
                            )
                    if q + 2 < 4:
                        nc.scalar.dma_start(
                            wqts[q + 2][:],
                            wfc_in[:, :, (q + 2) * 1024 : (q + 3) * 1024],
                        )
                for t in range(T2):
                    nc.vector.tensor_tensor(
                        out_sb[:, t, 0:512],
                        pss[t][:],
                        x2[t][:, 0:512],
                        op=ALU.add,
                    )
                # second pass: n=1 column half, t-major so the tail pipelines
                outv = out_dram[:].rearrange("(t q) c -> q t c", q=P)
                for t in range(T2):
                    ps2 = fc2ps.tile([P, 512], f32, tag="fc2", name=f"fc2b_{t}")
                    for f in range(FC):
                        nc.tensor.matmul(
                            ps2[:],
                            m1T[:, f, t * P : (t + 1) * P],
                            w2n[1][:, f, :],
                            start=(f == 0),
                            stop=(f == FC - 1),
                        )
                    nc.vector.tensor_tensor(
                        out_sb[:, t, 512:1024],
                        ps2[:],
                        x2[t][:, 512:1024],
                        op=ALU.add,
                    )
                    if use_bfc2:
                        nc.vector.tensor_tensor(
                            out_sb[:, t, :], out_sb[:, t, :], bfc2_bc[:],
                            op=ALU.add,
                        )
                    nc.sync.dma_start(outv[:, t, :], out_sb[:, t, :])

    nc.compile()
    return nc


def _get_program(flags):
    if flags not in _PROGRAM_CACHE:
        _PROGRAM_CACHE[flags] = _build_program(flags)
    return _PROGRAM_CACHE[flags]


def kernel(
    x,
    ln1_w,
    ln1_b,
    Wqkv,
    bqkv,
    Wproj,
    bproj,
    Er,
    ln2_w,
    ln2_b,
    Wfc,
    bfc,
    Wfc2,
    bfc2,
):
    import ml_dtypes
    from concourse.bass_utils import run_bass_kernel_spmd

    bf = ml_dtypes.bfloat16
    x = np.asarray(x, np.float32)
    f = np.float32
    ntriv = lambda a, v: not np.all(np.asarray(a) == v)
    flags = (
        ntriv(ln1_w, 1) or ntriv(ln1_b, 0),
        ntriv(ln2_w, 1) or ntriv(ln2_b, 0),
        ntriv(bqkv[:D], 0),
        ntriv(bqkv[D : 2 * D], 0),
        ntriv(bqkv[2 * D :], 0),
        ntriv(bproj, 0),
        ntriv(bfc, 0),
        ntriv(bfc2, 0),
    )
    nc = _get_program(flags)

    c = np.ascontiguousarray

    def pack_w(m, nch):
        # [rows, cols] -> [128, nch, cols] where rows = nch*128 chunk-major
        m = np.asarray(m, f)
        rows, cols = m.shape
        return c(m.reshape(nch, P, cols).transpose(1, 0, 2).astype(bf))

    ert2_f = np.concatenate([np.asarray(Er, f).T, np.asarray(Er, f).T], axis=0)
    ert2_pk = c(ert2_f.astype(bf))
    wfc_pk = pack_w(np.asarray(Wfc), DCH)
    wfc2_pk = pack_w(np.asarray(Wfc2), FC)

    in_maps = []
    for core in range(8):
        b, half = divmod(core, 2)
        hs0, hs1 = half * 512, (half + 1) * 512
        bq = np.asarray(bqkv[:D][hs0:hs1], f) * SCALE
        bk = np.asarray(bqkv[D : 2 * D][hs0:hs1], f)
        wq = np.asarray(Wqkv)[:, 0:D][:, hs0:hs1] * SCALE
        x_r = x[b].reshape(TC, P, D)
        x_pk = c(x_r.transpose(1, 0, 2).astype(bf))
        xmy_pk = c(x_r[half * T2 : (half + 1) * T2].transpose(1, 0, 2).astype(bf))
        in_maps.append(
            {
                "x": x_pk,
                "x_my": xmy_pk,
                "wq": pack_w(wq, DCH),
                "wk": pack_w(np.asarray(Wqkv)[:, D : 2 * D][:, hs0:hs1], DCH),
                "wv": pack_w(np.asarray(Wqkv)[:, 2 * D :][:, hs0:hs1], DCH),
                "wproj": pack_w(np.asarray(Wproj)[hs0:hs1, :].reshape(4, 2, HS, D)[:, ::-1].reshape(512, D), 4),
                "ert2": ert2_pk,
                "wfc": wfc_pk,
                "wfc2": wfc2_pk,
                "ln1a": c(np.asarray(ln1_w), f),
                "ln1b": c(np.asarray(ln1_b), f),
                "ln2a": c(np.asarray(ln2_w), f),
                "ln2b": c(np.asarray(ln2_b), f),
                "bq": c(bq.reshape(4, P).T, f),
                "bk": c(bk.reshape(4, P).T, f),
                "bv": c(np.asarray(bqkv[2 * D :][hs0:hs1]), f),
                "bproj": c(np.asarray(bproj), f),
                "bfc": c(np.asarray(bfc).reshape(FC, P).T, f),
                "bfc2": c(np.asarray(bfc2), f),
            }
        )

    trace = bool(int(os.environ.get("KERNEL_TRACE", "0")))
    res = run_bass_kernel_spmd(nc, in_maps, list(range(8)), trace=trace)
    global LAST_EXEC_NS, LAST_RESULT
    LAST_EXEC_NS = res.exec_time_ns
    LAST_RESULT = res
    out = np.empty((B, L, D), np.float32)
    for core in range(8):
        b, half = divmod(core, 2)
        out[b, half * 512 : (half + 1) * 512] = res.results[core]["out_my"]
    return out


LAST_EXEC_NS = None
LAST_RESULT = None


# revision 97
# speedup vs baseline: 1.0175x; 1.0028x over previous
"""Trainium2 Bass kernel for nn_BlockWithCache (Music-Transformer block w/ rel-pos).

Sharding (8 NeuronCores, uniform SPMD program; per-core differences live in the
input data only):
  - core c: batch element b = c//2, tensor-parallel half = c%2.
  - Attention: TP over heads — each core computes its 8 of 16 heads for the
    full 1024-token sequence (weight column slices supplied by the host).
  - Wproj row-slices produce partial attention outputs; a pairwise
    ReduceScatter(add) both completes the sum and splits tokens in half.
  - From the residual on: token-split — each core owns 512 tokens through
    LN2 + FFN (full 4*D hidden) and writes a disjoint output half.

v2 notes:
  - bf16 everywhere on the matmul path; weights host-packed into SBUF tile
    layouts so each matrix loads with 1-3 large contiguous DMAs.
  - Transposed attention: logits are computed as [key, query] (kt stationary,
    qt moving); Srel (read back from the DRAM skew buffer in [q, k] rows) is
    accumulated into the same PSUM block by a matmul with Srel as the
    stationary operand, which transposes it for free.  exp() then writes
    attT directly - no PE transposes and no separate PSUM->SBUF copies.
  - Softmax denominators come from a ones-column appended to V (attV PSUM row
    64); normalization is folded into the yp->ysb copy as a broadcast mult.
  - Skew/negpad DMAs issue from the (otherwise idle) Pool engine, bypassing
    the shared HWDGE descriptor-generation bottleneck.
  - FFN runs in two 256-token halves so FFN2(half A) overlaps FFN1(half B).
"""

import os
import sys

os.environ.setdefault("MYCRO_LOCAL_CACHE", "1")
if "/opt/trn_rl_repo" not in sys.path:
    sys.path.insert(0, "/opt/trn_rl_repo")

import numpy as np

B, L, D, H = 4, 1024, 1024, 16
HS = D // H          # 64
P = 128
TC = L // P          # 8 token chunks
DCH = D // P         # 8 feature chunks
NHC = H // 2         # 8 heads per core
FD = 4 * D           # 4096
FC = FD // P         # 32
TMY = L // 2         # 512 tokens owned after RS
T2 = TMY // P        # 4
EPS = 1e-5
SCALE = 1.0 / 8.0    # 1/sqrt(HS)
NEG = -1.0e9

_PROGRAM_CACHE = {}


def _build_program(flags, no_rs=False):
    import concourse.mybir as mybir
    import concourse.tile as tile
    from concourse import bacc
    from concourse.masks import make_identity

    (aff1, aff2, use_bq, use_bk, use_bv, use_bproj, use_bfc, use_bfc2) = flags

    f32 = mybir.dt.float32
    bf16 = mybir.dt.bfloat16
    fp8 = mybir.dt.float8e4
    AF = mybir.ActivationFunctionType
    ALU = mybir.AluOpType
    AX = mybir.AxisListType

    nc = bacc.Bacc("TRN2", target_bir_lowering=False, debug=False, num_devices=8)

    # Host-packed parameters (already in SBUF tile layout; see kernel()).
    x_in = nc.declare_dram_parameter("x", [P, TC, D], bf16, isOutput=False)
    xmy_in = nc.declare_dram_parameter("x_my", [P, T2, D], bf16, isOutput=False)
    wq_in = nc.declare_dram_parameter("wq", [P, DCH, 512], bf16, isOutput=False)
    wk_in = nc.declare_dram_parameter("wk", [P, DCH, 512], bf16, isOutput=False)
    wv_in = nc.declare_dram_parameter("wv", [P, DCH, 512], bf16, isOutput=False)
    wproj_in = nc.declare_dram_parameter("wproj", [P, 4, D], bf16, isOutput=False)
    ert2_in = nc.declare_dram_parameter("ert2", [P, L], bf16, isOutput=False)
    wfc_in = nc.declare_dram_parameter("wfc", [P, DCH, FD], bf16, isOutput=False)
    wfc2_in = nc.declare_dram_parameter("wfc2", [P, FC, D], bf16, isOutput=False)
    # Always-declared small params (cheap; used only when flags set)
    ln1a_in = nc.declare_dram_parameter("ln1a", [D], f32, isOutput=False)
    ln1b_in = nc.declare_dram_parameter("ln1b", [D], f32, isOutput=False)
    ln2a_in = nc.declare_dram_parameter("ln2a", [D], f32, isOutput=False)
    ln2b_in = nc.declare_dram_parameter("ln2b", [D], f32, isOutput=False)
    bq_in = nc.declare_dram_parameter("bq", [P, 4], f32, isOutput=False)
    bk_in = nc.declare_dram_parameter("bk", [P, 4], f32, isOutput=False)
    bv_in = nc.declare_dram_parameter("bv", [NHC * HS], f32, isOutput=False)
    bproj_in = nc.declare_dram_parameter("bproj", [D], f32, isOutput=False)
    bfc_in = nc.declare_dram_parameter("bfc", [P, FC], f32, isOutput=False)
    bfc2_in = nc.declare_dram_parameter("bfc2", [D], f32, isOutput=False)

    out_dram = nc.declare_dram_parameter("out_my", [TMY, D], f32, isOutput=True)

    def layernorm(tc, nc, pools, xin, hs, nchunks, aff, wbc, bbc, eps_ap,
                  sts=None):
        """Per-chunk two-pass LN; xin(t) returns the [P, D] f32 input AP,
        hs[t] is the bf16 output tile.  If sts is given, sts[t] is a [P, 8]
        stats tile whose col 0 already holds the row sums."""
        small, scratch = pools
        for t in range(nchunks):
            xap = xin(t)
            if sts is not None:
                st = sts[t]
            else:
                st = small.tile([P, 8], f32, tag="ln_st")
                # st cols: 0 sum, 1 sumsq, 2 mu, 3 mu2, 4 var, 5 std, 6 rstd, 7 mur
                nc.vector.reduce_sum(st[:, 0:1], xap, axis=AX.X)
            sq = scratch.tile([P, D], f32, tag="ln_sq")
            nc.scalar.activation(sq[:], xap, AF.Square, accum_out=st[:, 1:2])
            nc.vector.tensor_scalar_mul(st[:, 2:3], st[:, 0:1], 1.0 / D)
            nc.vector.tensor_tensor(st[:, 3:4], st[:, 2:3], st[:, 2:3], op=ALU.mult)
            nc.vector.tensor_scalar(
                st[:, 4:5], st[:, 1:2], 1.0 / D, st[:, 3:4],
                op0=ALU.mult, op1=ALU.subtract,
            )
            nc.scalar.activation(st[:, 5:6], st[:, 4:5], AF.Sqrt, bias=eps_ap)
            nc.vector.reciprocal(st[:, 6:7], st[:, 5:6])
            nc.vector.tensor_tensor(st[:, 7:8], st[:, 2:3], st[:, 6:7], op=ALU.mult)
            nc.vector.tensor_scalar(
                hs[t][:],
                xap,
                st[:, 6:7],
                st[:, 7:8],
                op0=ALU.mult,
                op1=ALU.subtract,
            )
            if aff:
                nc.vector.tensor_tensor(hs[t][:], hs[t][:], wbc[:], op=ALU.mult)
                nc.vector.tensor_tensor(hs[t][:], hs[t][:], bbc[:], op=ALU.add)

    with tile.TileContext(nc) as tc:
        import contextlib

        with contextlib.ExitStack() as es:
            cst = es.enter_context(tc.tile_pool(name="cst", bufs=1))
            small = es.enter_context(tc.tile_pool(name="small", bufs=2))
            dram = es.enter_context(tc.tile_pool(name="dram", bufs=1, space="DRAM"))

            eps_t = cst.tile([P, 1], f32)
            nc.vector.memset(eps_t[:], EPS)
            warm = cst.tile([P, 2], f32)
            nc.vector.memset(warm[:], 1.0)
            for fn in (AF.Square, AF.Sqrt, AF.Exp, AF.Gelu, AF.Copy):
                nc.scalar.activation(warm[:, 1:2], warm[:, 0:1], fn)
            id16 = cst.tile([P, P], bf16)
            make_identity(nc, id16)
            ert2 = cst.tile([P, L], bf16)
            nc.sync.dma_start(ert2[:], ert2_in[:])

            ln1w_bc = ln1b_bc = ln2w_bc = ln2b_bc = None
            if aff1:
                row = cst.tile([1, D], f32, tag="lnrow1a")
                nc.sync.dma_start(row[:], ln1a_in[None, :])
                ln1w_bc = cst.tile([P, D], f32)
                nc.gpsimd.partition_broadcast(ln1w_bc[:], row[:])
                row2 = cst.tile([1, D], f32, tag="lnrow1b")
                nc.sync.dma_start(row2[:], ln1b_in[None, :])
                ln1b_bc = cst.tile([P, D], f32)
                nc.gpsimd.partition_broadcast(ln1b_bc[:], row2[:])
            if aff2:
                row = cst.tile([1, D], f32, tag="lnrow2a")
                nc.sync.dma_start(row[:], ln2a_in[None, :])
                ln2w_bc = cst.tile([P, D], f32)
                nc.gpsimd.partition_broadcast(ln2w_bc[:], row[:])
                row2 = cst.tile([1, D], f32, tag="lnrow2b")
                nc.sync.dma_start(row2[:], ln2b_in[None, :])
                ln2b_bc = cst.tile([P, D], f32)
                nc.gpsimd.partition_broadcast(ln2b_bc[:], row2[:])
            bq_sb = bk_sb = None
            if use_bq:
                bq_sb = cst.tile([P, 4], f32)
                nc.sync.dma_start(bq_sb[:], bq_in[:])
            if use_bk:
                bk_sb = cst.tile([P, 4], f32)
                nc.sync.dma_start(bk_sb[:], bk_in[:])
            bv_bc = None
            if use_bv:
                row = cst.tile([1, NHC * HS], f32, tag="bvrow")
                nc.sync.dma_start(row[:], bv_in[None, :])
                bv_bc = cst.tile([P, NHC * HS], f32)
                nc.gpsimd.partition_broadcast(bv_bc[:], row[:])
            bproj_bc = None
            if use_bproj:
                row = cst.tile([1, D], f32, tag="bprow")
                nc.sync.dma_start(row[:], bproj_in[None, :])
                bproj_bc = cst.tile([P, D], f32)
                nc.gpsimd.partition_broadcast(bproj_bc[:], row[:])
            bfc_sb = None
            if use_bfc:
                bfc_sb = cst.tile([P, FC], f32)
                nc.sync.dma_start(bfc_sb[:], bfc_in[:])
            bfc2_bc = None
            if use_bfc2:
                row = cst.tile([1, D], f32, tag="b2row")
                nc.sync.dma_start(row[:], bfc2_in[None, :])
                bfc2_bc = cst.tile([P, D], f32)
                nc.gpsimd.partition_broadcast(bfc2_bc[:], row[:])

            # Skew DRAM buffers: per (qc, pr-parity): [2 slots][128 rows][srow]
            # bf16.  Write rows at stride srow, read back at stride srow-1 =>
            # row q is realigned by (127 - q); pad cols [wp, wp+128) hold NEG
            # so the causal mask comes back for free.
            negpad = cst.tile([P, 2, P], bf16)
            nc.vector.memset(negpad[:], NEG)
            skewbufs = []
            for qc in range(TC):
                srow = P * (qc + 2)
                slots = []
                for par in range(2):
                    d1 = dram.tile([2 * P * srow], bf16, name=f"skew_{qc}_{par}")
                    slots.append(d1)
                skewbufs.append(slots)

            def emit_negpads():
                for qc in range(TC):
                    srow = P * (qc + 2)
                    wp = P * (qc + 1)
                    for par in range(2):
                        d1 = skewbufs[qc][par]
                        wv_full = d1[:].rearrange(
                            "(s q c) -> q s c", s=2, q=P, c=srow
                        )
                        nc.sync.dma_start(wv_full[:, :, wp : wp + P], negpad[:])

            # ---------------- persistent activation tiles ----------------
            xmyp = es.enter_context(tc.tile_pool(name="xmyp", bufs=1))
            xmy = xmyp.tile([P, T2, D], bf16)
            wfcq_pool = es.enter_context(tc.tile_pool(name="wfcq", bufs=1))
            wqts = [
                wfcq_pool.tile([P, DCH, 1024], bf16, tag=f"wfcq{q % 2}", name=f"wqt{q}")
                for q in range(4)
            ]
            ysb_pool = tc.alloc_tile_pool(name="ysb", bufs=1)
            ysb = ysb_pool.tile([P, 4, L], bf16)

            qkv_pool = tc.alloc_tile_pool(name="qkv", bufs=1)
            qt_sb = [qkv_pool.tile([P, L], bf16, name=f"qt{p}") for p in range(4)]
            kt_sb = [qkv_pool.tile([P, L], bf16, name=f"kt{p}") for p in range(4)]
            # V with a ones column per head: [:, h, 0:64] = V, [:, h, 64] = 1
            v_sb = [qkv_pool.tile([P, NHC, HS + 1], bf16, name=f"v{t}") for t in range(TC)]

            # ---------------- LN1 + transpose + QKV ----------------
            with tc.tile_pool(name="xp", bufs=1) as xph, tc.tile_pool(
                name="hTp", bufs=1
            ) as hTp:
                xs = xph.tile([P, TC, D], bf16)
                for lo, hi in ((0, 1), (1, 2), (2, 4), (4, 6), (6, 8)):
                    nc.sync.dma_start(xs[:, lo:hi, :], x_in[:, lo:hi, :])
                hTT = hTp.tile([P, DCH, L], bf16)
                with tc.tile_pool(name="xh", bufs=1) as xh, tc.tile_pool(
                    name="lnscr", bufs=3
                ) as lnscr:
                    hs = [xh.tile([P, D], bf16, name=f"h{t}") for t in range(TC)]
                    layernorm(
                        tc, nc, (small, lnscr), lambda t: xs[:, t, :], hs, TC,
                        aff1, ln1w_bc, ln1b_bc, eps_t[:],
                    )
                    with tc.tile_pool(name="htps", bufs=3, space="PSUM") as htps:
                        for t in range(TC):
                            tp = htps.tile([P, DCH, P], bf16, tag="htp")
                            for d in range(DCH):
                                nc.tensor.transpose(
                                    tp[:, d, :], hs[t][:, d * P : (d + 1) * P], id16[:]
                                )
                            nc.any.tensor_copy(hTT[:, :, t * P : (t + 1) * P], tp[:])

                # QKV projections (h freed; hTT alive)
                with tc.tile_pool(name="wqkv", bufs=1) as wp_pool, tc.tile_pool(
                    name="qkvps", bufs=5, space="PSUM"
                ) as qps:
                    wq_sb = wp_pool.tile([P, DCH, 512], bf16)
                    wk_sb = wp_pool.tile([P, DCH, 512], bf16)
                    wv_sb = wp_pool.tile([P, DCH, 512], bf16)
                    nc.sync.dma_start(wq_sb[:], wq_in[:])
                    nc.sync.dma_start(wk_sb[:], wk_in[:])
                    nc.sync.dma_start(wv_sb[:], wv_in[:])
                    emit_negpads()
                    # Q^T and K^T: out [128(2 heads), tokens]
                    for p in range(4):
                        for n in range(2):
                            ps = qps.tile([P, 512], f32, tag="qkvp")
                            for d in range(DCH):
                                nc.tensor.matmul(
                                    ps[:],
                                    wq_sb[:, d, p * P : (p + 1) * P],
                                    hTT[:, d, n * 512 : (n + 1) * 512],
                                    start=(d == 0),
                                    stop=(d == DCH - 1),
                                )
                            nc.any.tensor_copy(
                                qt_sb[p][:, n * 512 : (n + 1) * 512], ps[:]
                            )
                            if use_bq:
                                nc.vector.tensor_scalar_add(
                                    qt_sb[p][:, n * 512 : (n + 1) * 512],
                                    qt_sb[p][:, n * 512 : (n + 1) * 512],
                                    bq_sb[:, p : p + 1],
                                )
                        for n in range(2):
                            ps = qps.tile([P, 512], f32, tag="qkvp")
                            for d in range(DCH):
                                nc.tensor.matmul(
                                    ps[:],
                                    wk_sb[:, d, p * P : (p + 1) * P],
                                    hTT[:, d, n * 512 : (n + 1) * 512],
                                    start=(d == 0),
                                    stop=(d == DCH - 1),
                                )
                            nc.any.tensor_copy(
                                kt_sb[p][:, n * 512 : (n + 1) * 512], ps[:]
                            )
                            if use_bk:
                                nc.vector.tensor_scalar_add(
                                    kt_sb[p][:, n * 512 : (n + 1) * 512],
                                    kt_sb[p][:, n * 512 : (n + 1) * 512],
                                    bk_sb[:, p : p + 1],
                                )
                    # V: out [tokens, 512 hs-cols] -> strided into v_sb + ones
                    for t in range(TC):
                        ps = qps.tile([P, 512], f32, tag="qkvp")
                        for d in range(DCH):
                            nc.tensor.matmul(
                                ps[:],
                                hTT[:, d, t * P : (t + 1) * P],
                                wv_sb[:, d, :],
                                start=(d == 0),
                                stop=(d == DCH - 1),
                            )
                        if use_bv:
                            nc.vector.tensor_tensor(
                                ps[:], ps[:], bv_bc[:], op=ALU.add
                            )
                        nc.any.tensor_copy(v_sb[t][:, :, 0:HS], ps[:])
                        nc.vector.memset(v_sb[t][:, :, HS : HS + 1], 1.0)

            # ---------------- attention (transposed logits) ----------------
            nc.scalar.dma_start(xmy[:], xmy_in[:])
            for q in range(2):
                nc.scalar.dma_start(
                    wqts[q][:], wfc_in[:, :, q * 1024 : (q + 1) * 1024]
                )
            with contextlib.ExitStack() as att_es:
                srelp = att_es.enter_context(tc.tile_pool(name="srelp", bufs=2))
                rsbp = att_es.enter_context(tc.tile_pool(name="rsbp", bufs=4))
                attTp = att_es.enter_context(tc.tile_pool(name="attTp", bufs=3))
                y1p = att_es.enter_context(tc.tile_pool(name="y1p", bufs=3))
                nrmp = att_es.enter_context(tc.tile_pool(name="nrmp", bufs=4))
                lps = att_es.enter_context(tc.tile_pool(name="lps", bufs=3, space="PSUM"))
                rps = att_es.enter_context(tc.tile_pool(name="rps", bufs=3, space="PSUM"))
                yps = att_es.enter_context(tc.tile_pool(name="yps", bufs=2, space="PSUM"))

                def emit_rphase(pr):
                    """R = Q Er^T -> DRAM skew write -> skewed read (Srel).
                    Both heads (slots) of the pair in one pass."""
                    srels = []
                    for qc in range(TC):
                        wp = P * (qc + 1)
                        m0 = 896 - P * qc
                        srow = P * (qc + 2)
                        nsub = (wp + 511) // 512
                        d1 = skewbufs[qc][pr % 2]
                        wview = d1[:].rearrange("(s q c) -> q s c", s=2, q=P, c=srow)
                        rsb = rsbp.tile([P, 2, wp], bf16, tag="rsb")
                        for i in range(2):
                            off = i * HS
                            lhsq = qt_sb[pr][off : off + HS, qc * P : (qc + 1) * P]
                            for s in range(nsub):
                                w = min(512, wp - s * 512)
                                rp = rps.tile([P, 512], f32, tag="rp")
                                nc.tensor.matmul(
                                    rp[:, :w],
                                    lhsq,
                                    ert2[off : off + HS, m0 + s * 512 : m0 + s * 512 + w],
                                    start=True,
                                    stop=True,
                                )
                                nc.vector.tensor_copy(
                                    rsb[:, i, s * 512 : s * 512 + w], rp[:, :w]
                                )
                        nc.gpsimd.dma_start(wview[:, :, :wp], rsb[:])
                        srel = srelp.tile([P, 2, wp], bf16, tag=f"srel{qc}")
                        for i in range(2):
                            rv = d1[i * P * srow + 127 : i * P * srow + 127 + P * (srow - 1)]
                            rview = rv.rearrange("(q c) -> q c", c=srow - 1)
                            nc.sync.dma_start(srel[:, i, :], rview[:, :wp])
                        srels.append(srel)
                    return srels

                srel_pending = {0: emit_rphase(0)}
                for pr in range(4):
                    if pr + 1 < 4:
                        srel_pending[pr + 1] = emit_rphase(pr + 1)
                    srels2 = srel_pending.pop(pr)
                    attT2 = [
                        attTp.tile([P, TC, L], bf16, tag="attT", name=f"attT_{pr}_{i}")
                        for i in range(2)
                    ]
                    # logits^T blocks + exp
                    for qc in range(TC):
                        for i in range(2):
                            off = i * HS
                            qmov = qt_sb[pr][off : off + HS, qc * P : (qc + 1) * P]
                            for cg in range(0, qc + 1, 4):
                                ncc = min(4, qc + 1 - cg)
                                lt = lps.tile([P, 4, P], f32, tag="lt")
                                for j in range(ncc):
                                    cc = cg + j
                                    nc.tensor.matmul(
                                        lt[:, j, :],
                                        kt_sb[pr][off : off + HS, cc * P : (cc + 1) * P],
                                        qmov,
                                        start=True,
                                        stop=False,
                                    )
                                    nc.tensor.matmul(
                                        lt[:, j, :],
                                        srels2[qc][:, i, cc * P : (cc + 1) * P],
                                        id16[:],
                                        start=False,
                                        stop=True,
                                    )
                                nc.scalar.activation(
                                    attT2[i][:, cg : cg + ncc, qc * P : (qc + 1) * P],
                                    lt[:, 0:ncc, :],
                                    AF.Exp,
                                )
                    # att @ V with ones-column -> y rows 0..63, denom row 64
                    h0 = 2 * pr
                    for i in range(2):
                        h = h0 + i
                        for n in range(2):
                            yp = yps.tile([P, 512], f32, tag="yp")
                            ccmax = min(TC, 4 * (n + 1))
                            for cc in range(ccmax):
                                lo = max(n * 512, cc * P)
                                w = (n + 1) * 512 - lo
                                nc.tensor.matmul(
                                    yp[0 : HS + 1, lo - n * 512 : lo - n * 512 + w],
                                    v_sb[cc][:, h, :],
                                    attT2[i][:, cc, lo : lo + w],
                                    start=(cc == 0),
                                    stop=(cc == ccmax - 1),
                                )
                            # normalize: rows 0..63 * (1 / row 64)
                            rcp = nrmp.tile([P, 512], f32, tag="rcp")
                            nrm = nrmp.tile([P, 512], f32, tag="nrm")
                            nc.vector.reciprocal(rcp[0:1, :], yp[HS : HS + 1, :])
                            nc.gpsimd.partition_broadcast(nrm[0:HS, :], rcp[0:1, :])
                            if i == 1:
                                nc.vector.tensor_tensor(
                                    ysb[0:HS, pr, n * 512 : (n + 1) * 512],
                                    yp[0:HS, :],
                                    nrm[0:HS, :],
                                    op=ALU.mult,
                                )
                            else:
                                y1 = y1p.tile([P, 512], bf16, tag="y1")
                                nc.vector.tensor_tensor(
                                    y1[0:HS, :],
                                    yp[0:HS, :],
                                    nrm[0:HS, :],
                                    op=ALU.mult,
                                )
                                nc.sync.dma_start(
                                    ysb[HS:P, pr, n * 512 : (n + 1) * 512],
                                    y1[0:HS, :],
                                )

            qkv_pool.release()

            # Internal DRAM for the pairwise ReduceScatter
            cc_in = [dram.tile([L, 512], bf16, name=f"cc_in{n}") for n in range(2)]
            cc_out = [dram.tile([TMY, 512], bf16, name=f"cc_out{n}") for n in range(2)]

            # ---------------- proj (partial) + ReduceScatter ----------------
            with tc.tile_pool(name="wproj", bufs=1) as wpp, tc.tile_pool(
                name="asb", bufs=1
            ) as asbp, tc.tile_pool(name="aps", bufs=4, space="PSUM") as apsp:
                wproj_sb = wpp.tile([P, 4, D], bf16)
                nc.scalar.dma_start(wproj_sb[:], wproj_in[:])
                for n in range(2):
                    asb = asbp.tile([P, TC, 512], bf16, tag=f"asb{n}")
                    for t in range(TC):
                        ap_ = apsp.tile([P, 512], f32, tag="ap")
                        for p in range(4):
                            nc.tensor.matmul(
                                ap_[:],
                                ysb[:, p, t * P : (t + 1) * P],
                                wproj_sb[:, p, n * 512 : (n + 1) * 512],
                                start=(p == 0),
                                stop=(p == 3),
                            )
                        nc.any.tensor_copy(asb[:, t, :], ap_[:])
                        if t % 2 == 1:
                            nc.sync.dma_start(
                                cc_in[n][:].rearrange("(t q) c -> q t c", q=P)[
                                    :, t - 1 : t + 1, :
                                ],
                                asb[:, t - 1 : t + 1, :],
                            )
                    # fire the column-half collective as soon as its inputs
                    # are written; the other half's matmuls overlap it
                    if no_rs:
                        nc.sync.dma_start(cc_out[n][:], cc_in[n][:TMY, :])
                    else:
                        nc.gpsimd.collective_compute(
                            "ReduceScatter",
                            mybir.AluOpType.add,
                            replica_groups=[[0, 1], [2, 3], [4, 5], [6, 7]],
                            ins=[cc_in[n][:]],
                            outs=[cc_out[n][:]],
                        )
            ysb_pool.release()

            # ---------------- residual + LN2 + h2T ----------------
            x2p = es.enter_context(tc.tile_pool(name="x2p", bufs=1))
            x2 = [x2p.tile([P, D], f32, name=f"x2_{t}") for t in range(T2)]
            h2Tp = es.enter_context(tc.tile_pool(name="h2Tp", bufs=1))
            h2TT = h2Tp.tile([P, DCH, TMY], bf16)
            with tc.tile_pool(name="res", bufs=1) as resp, tc.tile_pool(
                name="lnscr2", bufs=2
            ) as lnscr2:
                arb = resp.tile([P, T2, 2, 512], bf16, tag="arb")
                for n in range(2):
                    for g in range(2):
                        nc.sync.dma_start(
                            arb[:, 2 * g : 2 * g + 2, n, :],
                            cc_out[n][:].rearrange("(t q) c -> q t c", q=P)[
                                :, 2 * g : 2 * g + 2, :
                            ],
                        )
                h2 = [resp.tile([P, D], bf16, name=f"h2_{t}") for t in range(T2)]
                sts2 = [resp.tile([P, 8], f32, name=f"st2_{t}") for t in range(T2)]
                for t in range(T2):
                    if use_bproj:
                        nc.vector.tensor_tensor(
                            x2[t][:], xmy[:, t, :], arb[:, t, :, :], op=ALU.add
                        )
                        nc.vector.scalar_tensor_tensor(
                            x2[t][:], x2[t][:], 1.0, bproj_bc[:],
                            op0=ALU.mult, op1=ALU.add,
                            accum_out=sts2[t][:, 0:1],
                        )
                    else:
                        nc.vector.scalar_tensor_tensor(
                            x2[t][:], xmy[:, t, :], 1.0, arb[:, t, :, :],
                            op0=ALU.mult, op1=ALU.add,
                            accum_out=sts2[t][:, 0:1],
                        )
                layernorm(
                    tc, nc, (small, lnscr2), lambda t: x2[t][:], h2, T2,
                    aff2, ln2w_bc, ln2b_bc, eps_t[:], sts=sts2,
                )
                with tc.tile_pool(name="h2ps", bufs=2, space="PSUM") as h2ps:
                    for t in range(T2):
                        tp = h2ps.tile([P, DCH, P], bf16, tag="h2p")
                        for d in range(DCH):
                            nc.tensor.transpose(
                                tp[:, d, :], h2[t][:, d * P : (d + 1) * P], id16[:]
                            )
                        nc.any.tensor_copy(h2TT[:, :, t * P : (t + 1) * P], tp[:])

            # ---------------- FFN (f-streamed; FFN2 n=0 rides FFN1) ----------
            # FFN1 produces m1T[:, f, :] per f-chunk; FFN2's n=0 column half
            # accumulates in PSUM as each chunk lands, so PE stays dense.  The
            # n=1 half runs as a second f-pass from the kept m1T.
            m1p = es.enter_context(tc.tile_pool(name="m1p", bufs=1))
            m1T = m1p.tile([P, FC, TMY], bf16)
            outp = es.enter_context(tc.tile_pool(name="outp", bufs=1))
            out_sb = outp.tile([P, T2, D], f32)
            with tc.tile_pool(name="wfc2p", bufs=1) as wfc2_pool, tc.tile_pool(
                name="fc1ps", bufs=3, space="PSUM"
            ) as fc1ps, tc.tile_pool(name="fc2ps", bufs=4, space="PSUM") as fc2ps:
                w2n = [
                    wfc2_pool.tile([P, FC, 512], bf16, tag=f"w2n{n}", name=f"w2n{n}")
                    for n in range(2)
                ]
                nc.scalar.dma_start(w2n[0][:], wfc2_in[:, :, 0:512])
                nc.sync.dma_start(w2n[1][:], wfc2_in[:, :, 512:1024])
                pss = [fc2ps.tile([P, 512], f32, tag="fc2", name=f"fc2a_{t}") for t in range(T2)]
                for q in range(4):
                    wq_t = wqts[q]
                    for fl in range(DCH):
                        f = q * DCH + fl
                        mp = fc1ps.tile([P, TMY], f32, tag="m1ps")
                        for d in range(DCH):
                            nc.tensor.matmul(
                                mp[:],
                                wq_t[:, d, fl * P : (fl + 1) * P],
                                h2TT[:, d, :],
                                start=(d == 0),
                                stop=(d == DCH - 1),
                            )
                        if use_bfc:
                            nc.scalar.activation(
                                m1T[:, f, :], mp[:], AF.Gelu,
                                bias=bfc_sb[:, f : f + 1],
                            )
                        else:
                            nc.scalar.activation(m1T[:, f, :], mp[:], AF.Gelu)
                        for t in range(T2):
                            nc.tensor.matmul(
                                pss[t][:],
                                m1T[:, f, t * P : (t + 1) * P],
                                w2n[0][:, f, :],
                                start=(f == 0),
                                stop=(f == FC - 1),
                            )
                    if q + 2 < 4:
                        nc.scalar.dma_start(
                            wqts[q + 2][:],
                            wfc_in[:, :, (q + 2) * 1024 : (q + 3) * 1024],
                        )
                outv = out_dram[:].rearrange("(t q) c -> q t c", q=P)
                for t in range(T2):
                    nc.vector.tensor_tensor(
                        out_sb[:, t, 0:512],
                        pss[t][:],
                        x2[t][:, 0:512],
                        op=ALU.add,
                    )
                    nc.sync.dma_start(outv[:, t, 0:512], out_sb[:, t, 0:512])
                # second pass: n=1 column half, t-major so the tail pipelines
                for t in range(T2):
                    ps2 = fc2ps.tile([P, 512], f32, tag="fc2", name=f"fc2b_{t}")
                    for f in range(FC):
                        nc.tensor.matmul(
                            ps2[:],
                            m1T[:, f, t * P : (t + 1) * P],
                            w2n[1][:, f, :],
                            start=(f == 0),
                            stop=(f == FC - 1),
                        )
                    nc.vector.tensor_tensor(
                        out_sb[:, t, 512:1024],
                        ps2[:],
                        x2[t][:, 512:1024],
                        op=ALU.add,
                    )
                    if use_bfc2:
                        nc.vector.tensor_tensor(
                            out_sb[:, t, :], out_sb[:, t, :], bfc2_bc[:],
                            op=ALU.add,
                        )
                        nc.sync.dma_start(outv[:, t, :], out_sb[:, t, :])
                    else:
                        nc.sync.dma_start(
                            outv[:, t, 512:1024], out_sb[:, t, 512:1024]
                        )

    nc.compile()
    return nc


def _get_program(flags):
    if flags not in _PROGRAM_CACHE:
        _PROGRAM_CACHE[flags] = _build_program(flags)
    return _PROGRAM_CACHE[flags]


def kernel(
    x,
    ln1_w,
    ln1_b,
    Wqkv,
    bqkv,
    Wproj,
    bproj,
    Er,
    ln2_w,
    ln2_b,
    Wfc,
    bfc,
    Wfc2,
    bfc2,
):
    import ml_dtypes
    from concourse.bass_utils import run_bass_kernel_spmd

    bf = ml_dtypes.bfloat16
    x = np.asarray(x, np.float32)
    f = np.float32
    ntriv = lambda a, v: not np.all(np.asarray(a) == v)
    flags = (
        ntriv(ln1_w, 1) or ntriv(ln1_b, 0),
        ntriv(ln2_w, 1) or ntriv(ln2_b, 0),
        ntriv(bqkv[:D], 0),
        ntriv(bqkv[D : 2 * D], 0),
        ntriv(bqkv[2 * D :], 0),
        ntriv(bproj, 0),
        ntriv(bfc, 0),
        ntriv(bfc2, 0),
    )
    nc = _get_program(flags)

    c = np.ascontiguousarray

    def pack_w(m, nch):
        # [rows, cols] -> [128, nch, cols] where rows = nch*128 chunk-major
        m = np.asarray(m, f)
        rows, cols = m.shape
        return c(m.reshape(nch, P, cols).transpose(1, 0, 2).astype(bf))

    ert2_f = np.concatenate([np.asarray(Er, f).T, np.asarray(Er, f).T], axis=0)
    ert2_pk = c(ert2_f.astype(bf))
    wfc_pk = pack_w(np.asarray(Wfc), DCH)
    wfc2_pk = pack_w(np.asarray(Wfc2), FC)

    in_maps = []
    for core in range(8):
        b, half = divmod(core, 2)
        hs0, hs1 = half * 512, (half + 1) * 512
        bq = np.asarray(bqkv[:D][hs0:hs1], f) * SCALE
        bk = np.asarray(bqkv[D : 2 * D][hs0:hs1], f)
        wq = np.asarray(Wqkv)[:, 0:D][:, hs0:hs1] * SCALE
        x_r = x[b].reshape(TC, P, D)
        x_pk = c(x_r.transpose(1, 0, 2).astype(bf))
        xmy_pk = c(x_r[half * T2 : (half + 1) * T2].transpose(1, 0, 2).astype(bf))
        in_maps.append(
            {
                "x": x_pk,
                "x_my": xmy_pk,
                "wq": pack_w(wq, DCH),
                "wk": pack_w(np.asarray(Wqkv)[:, D : 2 * D][:, hs0:hs1], DCH),
                "wv": pack_w(np.asarray(Wqkv)[:, 2 * D :][:, hs0:hs1], DCH),
                "wproj": pack_w(np.asarray(Wproj)[hs0:hs1, :].reshape(4, 2, HS, D)[:, ::-1].reshape(512, D), 4),
                "ert2": ert2_pk,
                "wfc": wfc_pk,
                "wfc2": wfc2_pk,
                "ln1a": c(np.asarray(ln1_w), f),
                "ln1b": c(np.asarray(ln1_b), f),
                "ln2a": c(np.asarray(ln2_w), f),
                "ln2b": c(np.asarray(ln2_b), f),
                "bq": c(bq.reshape(4, P).T, f),
                "bk": c(bk.reshape(4, P).T, f),
                "bv": c(np.asarray(bqkv[2 * D :][hs0:hs1]), f),
                "bproj": c(np.asarray(bproj), f),
                "bfc": c(np.asarray(bfc).reshape(FC, P).T, f),
                "bfc2": c(np.asarray(bfc2), f),
            }
        )

    trace = bool(int(os.environ.get("KERNEL_TRACE", "0")))
    res = run_bass_kernel_spmd(nc, in_maps, list(range(8)), trace=trace)
    global LAST_EXEC_NS, LAST_RESULT
    LAST_EXEC_NS = res.exec_time_ns
    LAST_RESULT = res
    out = np.empty((B, L, D), np.float32)
    for core in range(8):
        b, half = divmod(core, 2)
        out[b, half * 512 : (half + 1) * 512] = res.results[core]["out_my"]
    return out


LAST_EXEC_NS = None
LAST_RESULT = None


# revision 106
# speedup vs baseline: 1.0208x; 1.0033x over previous
"""Trainium2 Bass kernel for nn_BlockWithCache (Music-Transformer block w/ rel-pos).

Sharding (8 NeuronCores, uniform SPMD program; per-core differences live in the
input data only):
  - core c: batch element b = c//2, tensor-parallel half = c%2.
  - Attention: TP over heads — each core computes its 8 of 16 heads for the
    full 1024-token sequence (weight column slices supplied by the host).
  - Wproj row-slices produce partial attention outputs; a pairwise
    ReduceScatter(add) both completes the sum and splits tokens in half.
  - From the residual on: token-split — each core owns 512 tokens through
    LN2 + FFN (full 4*D hidden) and writes a disjoint output half.

v2 notes:
  - bf16 everywhere on the matmul path; weights host-packed into SBUF tile
    layouts so each matrix loads with 1-3 large contiguous DMAs.
  - Transposed attention: logits are computed as [key, query] (kt stationary,
    qt moving); Srel (read back from the DRAM skew buffer in [q, k] rows) is
    accumulated into the same PSUM block by a matmul with Srel as the
    stationary operand, which transposes it for free.  exp() then writes
    attT directly - no PE transposes and no separate PSUM->SBUF copies.
  - Softmax denominators come from a ones-column appended to V (attV PSUM row
    64); normalization is folded into the yp->ysb copy as a broadcast mult.
  - Skew/negpad DMAs issue from the (otherwise idle) Pool engine, bypassing
    the shared HWDGE descriptor-generation bottleneck.
  - FFN runs in two 256-token halves so FFN2(half A) overlaps FFN1(half B).
"""

import os
import sys

os.environ.setdefault("MYCRO_LOCAL_CACHE", "1")
if "/opt/trn_rl_repo" not in sys.path:
    sys.path.insert(0, "/opt/trn_rl_repo")

import numpy as np

B, L, D, H = 4, 1024, 1024, 16
HS = D // H          # 64
P = 128
TC = L // P          # 8 token chunks
DCH = D // P         # 8 feature chunks
NHC = H // 2         # 8 heads per core
FD = 4 * D           # 4096
FC = FD // P         # 32
TMY = L // 2         # 512 tokens owned after RS
T2 = TMY // P        # 4
EPS = 1e-5
SCALE = 1.0 / 8.0    # 1/sqrt(HS)
NEG = -1.0e9

_PROGRAM_CACHE = {}


def _build_program(flags, no_rs=False):
    import concourse.mybir as mybir
    import concourse.tile as tile
    from concourse import bacc
    from concourse.masks import make_identity

    (aff1, aff2, use_bq, use_bk, use_bv, use_bproj, use_bfc, use_bfc2) = flags

    f32 = mybir.dt.float32
    bf16 = mybir.dt.bfloat16
    fp8 = mybir.dt.float8e4
    AF = mybir.ActivationFunctionType
    ALU = mybir.AluOpType
    AX = mybir.AxisListType

    nc = bacc.Bacc("TRN2", target_bir_lowering=False, debug=False, num_devices=8)

    # Host-packed parameters (already in SBUF tile layout; see kernel()).
    x_in = nc.declare_dram_parameter("x", [P, TC, D], bf16, isOutput=False)
    xmy_in = nc.declare_dram_parameter("x_my", [P, T2, D], bf16, isOutput=False)
    wq_in = nc.declare_dram_parameter("wq", [P, DCH, 512], bf16, isOutput=False)
    wk_in = nc.declare_dram_parameter("wk", [P, DCH, 512], bf16, isOutput=False)
    wv_in = nc.declare_dram_parameter("wv", [P, DCH, 512], bf16, isOutput=False)
    wproj_in = nc.declare_dram_parameter("wproj", [P, 4, D], bf16, isOutput=False)
    ert2_in = nc.declare_dram_parameter("ert2", [P, L], bf16, isOutput=False)
    wfc_in = nc.declare_dram_parameter("wfc", [P, DCH, FD], bf16, isOutput=False)
    wfc2_in = nc.declare_dram_parameter("wfc2", [P, FC, D], bf16, isOutput=False)
    # Always-declared small params (cheap; used only when flags set)
    ln1a_in = nc.declare_dram_parameter("ln1a", [D], f32, isOutput=False)
    ln1b_in = nc.declare_dram_parameter("ln1b", [D], f32, isOutput=False)
    ln2a_in = nc.declare_dram_parameter("ln2a", [D], f32, isOutput=False)
    ln2b_in = nc.declare_dram_parameter("ln2b", [D], f32, isOutput=False)
    bq_in = nc.declare_dram_parameter("bq", [P, 4], f32, isOutput=False)
    bk_in = nc.declare_dram_parameter("bk", [P, 4], f32, isOutput=False)
    bv_in = nc.declare_dram_parameter("bv", [NHC * HS], f32, isOutput=False)
    bproj_in = nc.declare_dram_parameter("bproj", [D], f32, isOutput=False)
    bfc_in = nc.declare_dram_parameter("bfc", [P, FC], f32, isOutput=False)
    bfc2_in = nc.declare_dram_parameter("bfc2", [D], f32, isOutput=False)

    out_dram = nc.declare_dram_parameter("out_my", [TMY, D], f32, isOutput=True)

    def layernorm(tc, nc, pools, xin, hs, nchunks, aff, wbc, bbc, eps_ap,
                  sts=None):
        """Per-chunk two-pass LN; xin(t) returns the [P, D] f32 input AP,
        hs[t] is the bf16 output tile.  If sts is given, sts[t] is a [P, 8]
        stats tile whose col 0 already holds the row sums."""
        small, scratch = pools
        for t in range(nchunks):
            xap = xin(t)
            if sts is not None:
                st = sts[t]
            else:
                st = small.tile([P, 8], f32, tag="ln_st")
                # st cols: 0 sum, 1 sumsq, 2 mu, 3 mu2, 4 var, 5 std, 6 rstd, 7 mur
                nc.vector.reduce_sum(st[:, 0:1], xap, axis=AX.X)
            sq = scratch.tile([P, D], f32, tag="ln_sq")
            nc.scalar.activation(sq[:], xap, AF.Square, accum_out=st[:, 1:2])
            nc.vector.tensor_scalar_mul(st[:, 2:3], st[:, 0:1], 1.0 / D)
            nc.vector.tensor_tensor(st[:, 3:4], st[:, 2:3], st[:, 2:3], op=ALU.mult)
            nc.vector.tensor_scalar(
                st[:, 4:5], st[:, 1:2], 1.0 / D, st[:, 3:4],
                op0=ALU.mult, op1=ALU.subtract,
            )
            nc.scalar.activation(st[:, 5:6], st[:, 4:5], AF.Sqrt, bias=eps_ap)
            nc.vector.reciprocal(st[:, 6:7], st[:, 5:6])
            nc.vector.tensor_tensor(st[:, 7:8], st[:, 2:3], st[:, 6:7], op=ALU.mult)
            nc.vector.tensor_scalar(
                hs[t][:],
                xap,
                st[:, 6:7],
                st[:, 7:8],
                op0=ALU.mult,
                op1=ALU.subtract,
            )
            if aff:
                nc.vector.tensor_tensor(hs[t][:], hs[t][:], wbc[:], op=ALU.mult)
                nc.vector.tensor_tensor(hs[t][:], hs[t][:], bbc[:], op=ALU.add)

    with tile.TileContext(nc) as tc:
        import contextlib

        with contextlib.ExitStack() as es:
            cst = es.enter_context(tc.tile_pool(name="cst", bufs=1))
            small = es.enter_context(tc.tile_pool(name="small", bufs=2))
            dram = es.enter_context(tc.tile_pool(name="dram", bufs=1, space="DRAM"))

            eps_t = cst.tile([P, 1], f32)
            nc.vector.memset(eps_t[:], EPS)
            warm = cst.tile([P, 2], f32)
            nc.vector.memset(warm[:], 1.0)
            for fn in (AF.Square, AF.Sqrt, AF.Exp, AF.Gelu, AF.Copy):
                nc.scalar.activation(warm[:, 1:2], warm[:, 0:1], fn)
            id16 = cst.tile([P, P], bf16)
            make_identity(nc, id16)
            ert2 = cst.tile([P, L], bf16)
            nc.sync.dma_start(ert2[:], ert2_in[:])

            ln1w_bc = ln1b_bc = ln2w_bc = ln2b_bc = None
            if aff1:
                row = cst.tile([1, D], f32, tag="lnrow1a")
                nc.sync.dma_start(row[:], ln1a_in[None, :])
                ln1w_bc = cst.tile([P, D], f32)
                nc.gpsimd.partition_broadcast(ln1w_bc[:], row[:])
                row2 = cst.tile([1, D], f32, tag="lnrow1b")
                nc.sync.dma_start(row2[:], ln1b_in[None, :])
                ln1b_bc = cst.tile([P, D], f32)
                nc.gpsimd.partition_broadcast(ln1b_bc[:], row2[:])
            if aff2:
                row = cst.tile([1, D], f32, tag="lnrow2a")
                nc.sync.dma_start(row[:], ln2a_in[None, :])
                ln2w_bc = cst.tile([P, D], f32)
                nc.gpsimd.partition_broadcast(ln2w_bc[:], row[:])
                row2 = cst.tile([1, D], f32, tag="lnrow2b")
                nc.sync.dma_start(row2[:], ln2b_in[None, :])
                ln2b_bc = cst.tile([P, D], f32)
                nc.gpsimd.partition_broadcast(ln2b_bc[:], row2[:])
            bq_sb = bk_sb = None
            if use_bq:
                bq_sb = cst.tile([P, 4], f32)
                nc.sync.dma_start(bq_sb[:], bq_in[:])
            if use_bk:
                bk_sb = cst.tile([P, 4], f32)
                nc.sync.dma_start(bk_sb[:], bk_in[:])
            bv_bc = None
            if use_bv:
                row = cst.tile([1, NHC * HS], f32, tag="bvrow")
                nc.sync.dma_start(row[:], bv_in[None, :])
                bv_bc = cst.tile([P, NHC * HS], f32)
                nc.gpsimd.partition_broadcast(bv_bc[:], row[:])
            bproj_bc = None
            if use_bproj:
                row = cst.tile([1, D], f32, tag="bprow")
                nc.sync.dma_start(row[:], bproj_in[None, :])
                bproj_bc = cst.tile([P, D], f32)
                nc.gpsimd.partition_broadcast(bproj_bc[:], row[:])
            bfc_sb = None
            if use_bfc:
                bfc_sb = cst.tile([P, FC], f32)
                nc.sync.dma_start(bfc_sb[:], bfc_in[:])
            bfc2_bc = None
            if use_bfc2:
                row = cst.tile([1, D], f32, tag="b2row")
                nc.sync.dma_start(row[:], bfc2_in[None, :])
                bfc2_bc = cst.tile([P, D], f32)
                nc.gpsimd.partition_broadcast(bfc2_bc[:], row[:])

            # Skew DRAM buffers: per (qc, pr-parity): [2 slots][128 rows][srow]
            # bf16.  Write rows at stride srow, read back at stride srow-1 =>
            # row q is realigned by (127 - q); pad cols [wp, wp+128) hold NEG
            # so the causal mask comes back for free.
            negpad = cst.tile([P, 2, P], bf16)
            nc.vector.memset(negpad[:], NEG)
            skewbufs = []
            for qc in range(TC):
                srow = P * (qc + 2)
                slots = []
                for par in range(2):
                    d1 = dram.tile([2 * P * srow], bf16, name=f"skew_{qc}_{par}")
                    slots.append(d1)
                skewbufs.append(slots)

            def emit_negpads():
                for qc in range(TC):
                    srow = P * (qc + 2)
                    wp = P * (qc + 1)
                    for par in range(2):
                        d1 = skewbufs[qc][par]
                        wv_full = d1[:].rearrange(
                            "(s q c) -> q s c", s=2, q=P, c=srow
                        )
                        nc.sync.dma_start(wv_full[:, :, wp : wp + P], negpad[:])

            # ---------------- persistent activation tiles ----------------
            xmyp = es.enter_context(tc.tile_pool(name="xmyp", bufs=1))
            xmy = xmyp.tile([P, T2, D], bf16)
            wfcq_pool = es.enter_context(tc.tile_pool(name="wfcq", bufs=1))
            wqts = [
                wfcq_pool.tile([P, DCH, 1024], bf16, tag=f"wfcq{q % 2}", name=f"wqt{q}")
                for q in range(4)
            ]
            ysb_pool = tc.alloc_tile_pool(name="ysb", bufs=1)
            ysb = ysb_pool.tile([P, 4, L], bf16)

            qkv_pool = tc.alloc_tile_pool(name="qkv", bufs=1)
            qt_sb = [qkv_pool.tile([P, L], bf16, name=f"qt{p}") for p in range(4)]
            kt_sb = [qkv_pool.tile([P, L], bf16, name=f"kt{p}") for p in range(4)]
            # V with a ones column per head: [:, h, 0:64] = V, [:, h, 64] = 1
            v_sb = [qkv_pool.tile([P, NHC, HS + 1], bf16, name=f"v{t}") for t in range(TC)]

            # ---------------- LN1 + transpose + QKV ----------------
            with tc.tile_pool(name="xp", bufs=1) as xph, tc.tile_pool(
                name="hTp", bufs=1
            ) as hTp:
                xs = xph.tile([P, TC, D], bf16)
                for lo, hi in ((0, 1), (1, 2), (2, 4), (4, 6), (6, 8)):
                    nc.sync.dma_start(xs[:, lo:hi, :], x_in[:, lo:hi, :])
                hTT = hTp.tile([P, DCH, L], bf16)
                with tc.tile_pool(name="xh", bufs=1) as xh, tc.tile_pool(
                    name="lnscr", bufs=3
                ) as lnscr:
                    hs = [xh.tile([P, D], bf16, name=f"h{t}") for t in range(TC)]
                    layernorm(
                        tc, nc, (small, lnscr), lambda t: xs[:, t, :], hs, TC,
                        aff1, ln1w_bc, ln1b_bc, eps_t[:],
                    )
                    with tc.tile_pool(name="htps", bufs=3, space="PSUM") as htps:
                        for t in range(TC):
                            tp = htps.tile([P, DCH, P], bf16, tag="htp")
                            for d in range(DCH):
                                nc.tensor.transpose(
                                    tp[:, d, :], hs[t][:, d * P : (d + 1) * P], id16[:]
                                )
                            nc.any.tensor_copy(hTT[:, :, t * P : (t + 1) * P], tp[:])

                # QKV projections (h freed; hTT alive)
                with tc.tile_pool(name="wqkv", bufs=1) as wp_pool, tc.tile_pool(
                    name="qkvps", bufs=5, space="PSUM"
                ) as qps:
                    wq_sb = wp_pool.tile([P, DCH, 512], bf16)
                    wk_sb = wp_pool.tile([P, DCH, 512], bf16)
                    wv_sb = wp_pool.tile([P, DCH, 512], bf16)
                    nc.sync.dma_start(wq_sb[:], wq_in[:])
                    nc.sync.dma_start(wk_sb[:], wk_in[:])
                    nc.sync.dma_start(wv_sb[:], wv_in[:])
                    emit_negpads()
                    # Q^T and K^T: out [128(2 heads), tokens]
                    for p in range(4):
                        for n in range(2):
                            ps = qps.tile([P, 512], f32, tag="qkvp")
                            for d in range(DCH):
                                nc.tensor.matmul(
                                    ps[:],
                                    wq_sb[:, d, p * P : (p + 1) * P],
                                    hTT[:, d, n * 512 : (n + 1) * 512],
                                    start=(d == 0),
                                    stop=(d == DCH - 1),
                                )
                            nc.any.tensor_copy(
                                qt_sb[p][:, n * 512 : (n + 1) * 512], ps[:]
                            )
                            if use_bq:
                                nc.vector.tensor_scalar_add(
                                    qt_sb[p][:, n * 512 : (n + 1) * 512],
                                    qt_sb[p][:, n * 512 : (n + 1) * 512],
                                    bq_sb[:, p : p + 1],
                                )
                        for n in range(2):
                            ps = qps.tile([P, 512], f32, tag="qkvp")
                            for d in range(DCH):
                                nc.tensor.matmul(
                                    ps[:],
                                    wk_sb[:, d, p * P : (p + 1) * P],
                                    hTT[:, d, n * 512 : (n + 1) * 512],
                                    start=(d == 0),
                                    stop=(d == DCH - 1),
                                )
                            nc.any.tensor_copy(
                                kt_sb[p][:, n * 512 : (n + 1) * 512], ps[:]
                            )
                            if use_bk:
                                nc.vector.tensor_scalar_add(
                                    kt_sb[p][:, n * 512 : (n + 1) * 512],
                                    kt_sb[p][:, n * 512 : (n + 1) * 512],
                                    bk_sb[:, p : p + 1],
                                )
                    # V: out [tokens, 512 hs-cols] -> strided into v_sb + ones
                    for t in range(TC):
                        ps = qps.tile([P, 512], f32, tag="qkvp")
                        for d in range(DCH):
                            nc.tensor.matmul(
                                ps[:],
                                hTT[:, d, t * P : (t + 1) * P],
                                wv_sb[:, d, :],
                                start=(d == 0),
                                stop=(d == DCH - 1),
                            )
                        if use_bv:
                            nc.vector.tensor_tensor(
                                ps[:], ps[:], bv_bc[:], op=ALU.add
                            )
                        nc.any.tensor_copy(v_sb[t][:, :, 0:HS], ps[:])
                        nc.vector.memset(v_sb[t][:, :, HS : HS + 1], 1.0)

            # ---------------- attention (transposed logits) ----------------
            nc.scalar.dma_start(xmy[:], xmy_in[:])
            for q in range(2):
                nc.scalar.dma_start(
                    wqts[q][:], wfc_in[:, :, q * 1024 : (q + 1) * 1024]
                )
            with contextlib.ExitStack() as att_es:
                srelp = att_es.enter_context(tc.tile_pool(name="srelp", bufs=2))
                rsbp = att_es.enter_context(tc.tile_pool(name="rsbp", bufs=4))
                attTp = att_es.enter_context(tc.tile_pool(name="attTp", bufs=3))
                y1p = att_es.enter_context(tc.tile_pool(name="y1p", bufs=3))
                nrmp = att_es.enter_context(tc.tile_pool(name="nrmp", bufs=4))
                lps = att_es.enter_context(tc.tile_pool(name="lps", bufs=3, space="PSUM"))
                rps = att_es.enter_context(tc.tile_pool(name="rps", bufs=3, space="PSUM"))
                yps = att_es.enter_context(tc.tile_pool(name="yps", bufs=2, space="PSUM"))

                def emit_rphase(pr):
                    """R = Q Er^T -> DRAM skew write -> skewed read (Srel).
                    Both heads (slots) of the pair in one pass."""
                    srels = []
                    for qc in range(TC):
                        wp = P * (qc + 1)
                        m0 = 896 - P * qc
                        srow = P * (qc + 2)
                        nsub = (wp + 511) // 512
                        d1 = skewbufs[qc][pr % 2]
                        wview = d1[:].rearrange("(s q c) -> q s c", s=2, q=P, c=srow)
                        rsb = rsbp.tile([P, 2, wp], bf16, tag="rsb")
                        for i in range(2):
                            off = i * HS
                            lhsq = qt_sb[pr][off : off + HS, qc * P : (qc + 1) * P]
                            for s in range(nsub):
                                w = min(512, wp - s * 512)
                                rp = rps.tile([P, 512], f32, tag="rp")
                                nc.tensor.matmul(
                                    rp[:, :w],
                                    lhsq,
                                    ert2[off : off + HS, m0 + s * 512 : m0 + s * 512 + w],
                                    start=True,
                                    stop=True,
                                )
                                nc.vector.tensor_copy(
                                    rsb[:, i, s * 512 : s * 512 + w], rp[:, :w]
                                )
                        nc.gpsimd.dma_start(wview[:, :, :wp], rsb[:])
                        srel = srelp.tile([P, 2, wp], bf16, tag=f"srel{qc}")
                        for i in range(2):
                            rv = d1[i * P * srow + 127 : i * P * srow + 127 + P * (srow - 1)]
                            rview = rv.rearrange("(q c) -> q c", c=srow - 1)
                            nc.sync.dma_start(srel[:, i, :], rview[:, :wp])
                        srels.append(srel)
                    return srels

                srel_pending = {0: emit_rphase(0)}
                for pr in range(4):
                    if pr + 1 < 4:
                        srel_pending[pr + 1] = emit_rphase(pr + 1)
                    srels2 = srel_pending.pop(pr)
                    attT2 = [
                        attTp.tile([P, TC, L], bf16, tag="attT", name=f"attT_{pr}_{i}")
                        for i in range(2)
                    ]
                    # logits^T blocks + exp
                    for qc in range(TC):
                        for i in range(2):
                            off = i * HS
                            qmov = qt_sb[pr][off : off + HS, qc * P : (qc + 1) * P]
                            for cg in range(0, qc + 1, 4):
                                ncc = min(4, qc + 1 - cg)
                                lt = lps.tile([P, 4, P], f32, tag="lt")
                                for j in range(ncc):
                                    cc = cg + j
                                    nc.tensor.matmul(
                                        lt[:, j, :],
                                        kt_sb[pr][off : off + HS, cc * P : (cc + 1) * P],
                                        qmov,
                                        start=True,
                                        stop=False,
                                    )
                                    nc.tensor.matmul(
                                        lt[:, j, :],
                                        srels2[qc][:, i, cc * P : (cc + 1) * P],
                                        id16[:],
                                        start=False,
                                        stop=True,
                                    )
                                nc.scalar.activation(
                                    attT2[i][:, cg : cg + ncc, qc * P : (qc + 1) * P],
                                    lt[:, 0:ncc, :],
                                    AF.Exp,
                                )
                    # att @ V with ones-column -> y rows 0..63, denom row 64
                    h0 = 2 * pr
                    for i in range(2):
                        h = h0 + i
                        for n in range(2):
                            yp = yps.tile([P, 512], f32, tag="yp")
                            ccmax = min(TC, 4 * (n + 1))
                            for cc in range(ccmax):
                                lo = max(n * 512, cc * P)
                                w = (n + 1) * 512 - lo
                                nc.tensor.matmul(
                                    yp[0 : HS + 1, lo - n * 512 : lo - n * 512 + w],
                                    v_sb[cc][:, h, :],
                                    attT2[i][:, cc, lo : lo + w],
                                    start=(cc == 0),
                                    stop=(cc == ccmax - 1),
                                )
                            # normalize: rows 0..63 * (1 / row 64)
                            rcp = nrmp.tile([P, 512], f32, tag="rcp")
                            nrm = nrmp.tile([P, 512], f32, tag="nrm")
                            nc.vector.reciprocal(rcp[0:1, :], yp[HS : HS + 1, :])
                            nc.gpsimd.partition_broadcast(nrm[0:HS, :], rcp[0:1, :])
                            if i == 1:
                                nc.vector.tensor_tensor(
                                    ysb[0:HS, pr, n * 512 : (n + 1) * 512],
                                    yp[0:HS, :],
                                    nrm[0:HS, :],
                                    op=ALU.mult,
                                )
                            else:
                                y1 = y1p.tile([P, 512], bf16, tag="y1")
                                nc.vector.tensor_tensor(
                                    y1[0:HS, :],
                                    yp[0:HS, :],
                                    nrm[0:HS, :],
                                    op=ALU.mult,
                                )
                                nc.sync.dma_start(
                                    ysb[HS:P, pr, n * 512 : (n + 1) * 512],
                                    y1[0:HS, :],
                                )

            qkv_pool.release()

            # Internal DRAM for the pairwise ReduceScatter
            cc_in = [dram.tile([L, 512], bf16, name=f"cc_in{n}") for n in range(2)]
            cc_out = [dram.tile([TMY, 512], bf16, name=f"cc_out{n}") for n in range(2)]

            # ---------------- proj (partial) + ReduceScatter ----------------
            with tc.tile_pool(name="wproj", bufs=1) as wpp, tc.tile_pool(
                name="asb", bufs=1
            ) as asbp, tc.tile_pool(name="aps", bufs=4, space="PSUM") as apsp:
                wproj_sb = wpp.tile([P, 4, D], bf16)
                nc.scalar.dma_start(wproj_sb[:], wproj_in[:])
                for n in range(2):
                    asb = asbp.tile([P, TC, 512], bf16, tag=f"asb{n}")
                    for t in range(TC):
                        ap_ = apsp.tile([P, 512], f32, tag="ap")
                        for p in range(4):
                            nc.tensor.matmul(
                                ap_[:],
                                ysb[:, p, t * P : (t + 1) * P],
                                wproj_sb[:, p, n * 512 : (n + 1) * 512],
                                start=(p == 0),
                                stop=(p == 3),
                            )
                        nc.any.tensor_copy(asb[:, t, :], ap_[:])
                        if t % 2 == 1:
                            nc.sync.dma_start(
                                cc_in[n][:].rearrange("(t q) c -> q t c", q=P)[
                                    :, t - 1 : t + 1, :
                                ],
                                asb[:, t - 1 : t + 1, :],
                            )
                    # fire the column-half collective as soon as its inputs
                    # are written; the other half's matmuls overlap it
                    if no_rs:
                        nc.sync.dma_start(cc_out[n][:], cc_in[n][:TMY, :])
                    else:
                        nc.gpsimd.collective_compute(
                            "ReduceScatter",
                            mybir.AluOpType.add,
                            replica_groups=[[0, 1], [2, 3], [4, 5], [6, 7]],
                            ins=[cc_in[n][:]],
                            outs=[cc_out[n][:]],
                        )
            ysb_pool.release()

            # ---------------- residual + LN2 + h2T ----------------
            x2p = es.enter_context(tc.tile_pool(name="x2p", bufs=1))
            x2 = [x2p.tile([P, D], f32, name=f"x2_{t}") for t in range(T2)]
            h2Tp = es.enter_context(tc.tile_pool(name="h2Tp", bufs=1))
            h2TT = h2Tp.tile([P, DCH, TMY], bf16)
            with tc.tile_pool(name="res", bufs=1) as resp, tc.tile_pool(
                name="lnscr2", bufs=2
            ) as lnscr2:
                arb = resp.tile([P, T2, 2, 512], bf16, tag="arb")
                for n in range(2):
                    for g in range(2):
                        nc.sync.dma_start(
                            arb[:, 2 * g : 2 * g + 2, n, :],
                            cc_out[n][:].rearrange("(t q) c -> q t c", q=P)[
                                :, 2 * g : 2 * g + 2, :
                            ],
                        )
                h2 = [resp.tile([P, D], bf16, name=f"h2_{t}") for t in range(T2)]
                sts2 = [resp.tile([P, 8], f32, name=f"st2_{t}") for t in range(T2)]
                for t in range(T2):
                    if use_bproj:
                        nc.vector.tensor_tensor(
                            x2[t][:], xmy[:, t, :], arb[:, t, :, :], op=ALU.add
                        )
                        nc.vector.scalar_tensor_tensor(
                            x2[t][:], x2[t][:], 1.0, bproj_bc[:],
                            op0=ALU.mult, op1=ALU.add,
                            accum_out=sts2[t][:, 0:1],
                        )
                    else:
                        nc.vector.scalar_tensor_tensor(
                            x2[t][:], xmy[:, t, :], 1.0, arb[:, t, :, :],
                            op0=ALU.mult, op1=ALU.add,
                            accum_out=sts2[t][:, 0:1],
                        )
                layernorm(
                    tc, nc, (small, lnscr2), lambda t: x2[t][:], h2, T2,
                    aff2, ln2w_bc, ln2b_bc, eps_t[:], sts=sts2,
                )
                with tc.tile_pool(name="h2ps", bufs=2, space="PSUM") as h2ps:
                    for t in range(T2):
                        tp = h2ps.tile([P, DCH, P], bf16, tag="h2p")
                        for d in range(DCH):
                            nc.tensor.transpose(
                                tp[:, d, :], h2[t][:, d * P : (d + 1) * P], id16[:]
                            )
                        nc.any.tensor_copy(h2TT[:, :, t * P : (t + 1) * P], tp[:])

            # ---------------- FFN (f-streamed; FFN2 n=0 rides FFN1) ----------
            # FFN1 produces m1T[:, f, :] per f-chunk; FFN2's n=0 column half
            # accumulates in PSUM as each chunk lands, so PE stays dense.  The
            # n=1 half runs as a second f-pass from the kept m1T.
            m1p = es.enter_context(tc.tile_pool(name="m1p", bufs=1))
            m1T = m1p.tile([P, FC, TMY], bf16)
            outp = es.enter_context(tc.tile_pool(name="outp", bufs=1))
            out_sb = outp.tile([P, T2, D], f32)
            with tc.tile_pool(name="wfc2p", bufs=1) as wfc2_pool, tc.tile_pool(
                name="fc1ps", bufs=3, space="PSUM"
            ) as fc1ps, tc.tile_pool(name="fc2ps", bufs=5, space="PSUM") as fc2ps:
                w2n = [
                    wfc2_pool.tile([P, FC, 512], bf16, tag=f"w2n{n}", name=f"w2n{n}")
                    for n in range(2)
                ]
                nc.scalar.dma_start(w2n[0][:], wfc2_in[:, :, 0:512])
                nc.sync.dma_start(w2n[1][:], wfc2_in[:, :, 512:1024])
                pss = [fc2ps.tile([P, 512], f32, tag="fc2", name=f"fc2a_{t}") for t in range(T2)]
                for q in range(4):
                    wq_t = wqts[q]
                    for fl in range(DCH):
                        f = q * DCH + fl
                        mp = fc1ps.tile([P, TMY], f32, tag="m1ps")
                        for d in range(DCH):
                            nc.tensor.matmul(
                                mp[:],
                                wq_t[:, d, fl * P : (fl + 1) * P],
                                h2TT[:, d, :],
                                start=(d == 0),
                                stop=(d == DCH - 1),
                            )
                        if use_bfc:
                            nc.scalar.activation(
                                m1T[:, f, :], mp[:], AF.Gelu,
                                bias=bfc_sb[:, f : f + 1],
                            )
                        else:
                            nc.scalar.activation(m1T[:, f, :], mp[:], AF.Gelu)
                        for t in range(T2):
                            nc.tensor.matmul(
                                pss[t][:],
                                m1T[:, f, t * P : (t + 1) * P],
                                w2n[0][:, f, :],
                                start=(f == 0),
                                stop=(f == FC - 1),
                            )
                    if q + 2 < 4:
                        nc.scalar.dma_start(
                            wqts[q + 2][:],
                            wfc_in[:, :, (q + 2) * 1024 : (q + 3) * 1024],
                        )
                outv = out_dram[:].rearrange("(t q) c -> q t c", q=P)
                for t in range(T2):
                    nc.vector.tensor_tensor(
                        out_sb[:, t, 0:512],
                        pss[t][:],
                        x2[t][:, 0:512],
                        op=ALU.add,
                    )
                    nc.sync.dma_start(outv[:, t, 0:512], out_sb[:, t, 0:512])
                # second pass: n=1 column half, t-major so the tail pipelines
                for t in range(T2):
                    ps2 = fc2ps.tile([P, 512], f32, tag="fc2", name=f"fc2b_{t}")
                    for f in range(FC):
                        nc.tensor.matmul(
                            ps2[:],
                            m1T[:, f, t * P : (t + 1) * P],
                            w2n[1][:, f, :],
                            start=(f == 0),
                            stop=(f == FC - 1),
                        )
                    nc.vector.tensor_tensor(
                        out_sb[:, t, 512:1024],
                        ps2[:],
                        x2[t][:, 512:1024],
                        op=ALU.add,
                    )
                    if use_bfc2:
                        nc.vector.tensor_tensor(
                            out_sb[:, t, :], out_sb[:, t, :], bfc2_bc[:],
                            op=ALU.add,
                        )
                        nc.sync.dma_start(outv[:, t, :], out_sb[:, t, :])
                    else:
                        nc.sync.dma_start(
                            outv[:, t, 512:1024], out_sb[:, t, 512:1024]
                        )

    nc.compile()
    return nc


def _get_program(flags):
    if flags not in _PROGRAM_CACHE:
        _PROGRAM_CACHE[flags] = _build_program(flags)
    return _PROGRAM_CACHE[flags]


def kernel(
    x,
    ln1_w,
    ln1_b,
    Wqkv,
    bqkv,
    Wproj,
    bproj,
    Er,
    ln2_w,
    ln2_b,
    Wfc,
    bfc,
    Wfc2,
    bfc2,
):
    import ml_dtypes
    from concourse.bass_utils import run_bass_kernel_spmd

    bf = ml_dtypes.bfloat16
    x = np.asarray(x, np.float32)
    f = np.float32
    ntriv = lambda a, v: not np.all(np.asarray(a) == v)
    flags = (
        ntriv(ln1_w, 1) or ntriv(ln1_b, 0),
        ntriv(ln2_w, 1) or ntriv(ln2_b, 0),
        ntriv(bqkv[:D], 0),
        ntriv(bqkv[D : 2 * D], 0),
        ntriv(bqkv[2 * D :], 0),
        ntriv(bproj, 0),
        ntriv(bfc, 0),
        ntriv(bfc2, 0),
    )
    nc = _get_program(flags)

    c = np.ascontiguousarray

    def pack_w(m, nch):
        # [rows, cols] -> [128, nch, cols] where rows = nch*128 chunk-major
        m = np.asarray(m, f)
        rows, cols = m.shape
        return c(m.reshape(nch, P, cols).transpose(1, 0, 2).astype(bf))

    ert2_f = np.concatenate([np.asarray(Er, f).T, np.asarray(Er, f).T], axis=0)
    ert2_pk = c(ert2_f.astype(bf))
    wfc_pk = pack_w(np.asarray(Wfc), DCH)
    wfc2_pk = pack_w(np.asarray(Wfc2), FC)

    in_maps = []
    for core in range(8):
        b, half = divmod(core, 2)
        hs0, hs1 = half * 512, (half + 1) * 512
        bq = np.asarray(bqkv[:D][hs0:hs1], f) * SCALE
        bk = np.asarray(bqkv[D : 2 * D][hs0:hs1], f)
        wq = np.asarray(Wqkv)[:, 0:D][:, hs0:hs1] * SCALE
        x_r = x[b].reshape(TC, P, D)
        x_pk = c(x_r.transpose(1, 0, 2).astype(bf))
        xmy_pk = c(x_r[half * T2 : (half + 1) * T2].transpose(1, 0, 2).astype(bf))
        in_maps.append(
            {
                "x": x_pk,
                "x_my": xmy_pk,
                "wq": pack_w(wq, DCH),
                "wk": pack_w(np.asarray(Wqkv)[:, D : 2 * D][:, hs0:hs1], DCH),
                "wv": pack_w(np.asarray(Wqkv)[:, 2 * D :][:, hs0:hs1], DCH),
                "wproj": pack_w(np.asarray(Wproj)[hs0:hs1, :].reshape(4, 2, HS, D)[:, ::-1].reshape(512, D), 4),
                "ert2": ert2_pk,
                "wfc": wfc_pk,
                "wfc2": wfc2_pk,
                "ln1a": c(np.asarray(ln1_w), f),
                "ln1b": c(np.asarray(ln1_b), f),
                "ln2a": c(np.asarray(ln2_w), f),
                "ln2b": c(np.asarray(ln2_b), f),
                "bq": c(bq.reshape(4, P).T, f),
                "bk": c(bk.reshape(4, P).T, f),
                "bv": c(np.asarray(bqkv[2 * D :][hs0:hs1]), f),
                "bproj": c(np.asarray(bproj), f),
                "bfc": c(np.asarray(bfc).reshape(FC, P).T, f),
                "bfc2": c(np.asarray(bfc2), f),
            }
        )

    trace = bool(int(os.environ.get("KERNEL_TRACE", "0")))
    res = run_bass_kernel_spmd(nc, in_maps, list(range(8)), trace=trace)
    global LAST_EXEC_NS, LAST_RESULT
    LAST_EXEC_NS = res.exec_time_ns
    LAST_RESULT = res
    out = np.empty((B, L, D), np.float32)
    for core in range(8):
        b, half = divmod(core, 2)
        out[b, half * 512 : (half + 1) * 512] = res.results[core]["out_my"]
    return out


LAST_EXEC_NS = None
LAST_RESULT = None
